# revision 20
# baseline (speedup 1.0000x reference)
"""Bahdanau attention forward on 8 Trainium2 NeuronCores.

Data-parallel over batch B=32: 4 batches per core, dense weights replicated,
no collectives.  Shapes hardcoded: B=32, T=2048, De=Dd=H=1024.

Math (per batch b):
    enc_p  = h_enc[b] @ We + be                  [T, H]
    dec_p  = h_dec[b] @ Wd + bd                  [H]
    score  = tanh(enc_p + dec_p) @ Wc + bc       [T]
    attn   = softmax(score)                      [T]
    ctx    = sum_t attn[t] * enc_p[t]            [H]

Implementation notes:
  - h_enc is pre-transposed on the host to [De, T] so the contraction dim
    (De) lands on SBUF partitions; tiles of it are the stationary matmul
    operand, We (natural [De, H]) is the moving one -> enc_p in natural
    [T, H] layout.  All large matmuls use float32r (full PE rate, ~1e-4
    rel err; fp32 would be 1/4 rate).
  - be is never added on-chip to enc_p: score gets it through the dec row
    (dec_full = dec_p + bd + be) and ctx gets it analytically at the end
    (sum(attn) == 1 so ctx = ctx_raw + be).
  - bc shifts softmax by a constant; folded into the Exp bias (exact).
  - score[t] = sum_h tanh(...)[t,h] * Wc[h] is a free-dim fused
    multiply+reduce on DVE (scalar_tensor_tensor), with Wc pre-broadcast
    to 128 partitions on the host.
  - softmax needs no max-subtraction: |score| <= sum|Wc| ~ 25, exp is
    safe in fp32.
  - ctx = sum over 16 T-tiles of matmul(lhsT=exp_col[128,1],
    rhs=enc_p_tile[128,512]) on the UNNORMALIZED exp; the 1/sum scale and
    the be add fuse into one DVE op on the [1,512] result.  exp runs
    per-chunk so the final softmax->ctx chain is short.
  - Emission is software-pipelined: softmax+ctx of batch b are emitted
    after batch b+1's first chunk so the PE never head-of-line blocks on
    the softmax chain; the first chunk of batch 0 defers its dec_p add
    so the PE does not wait for the Wd load at startup.
"""

import os
import sys
import types

import numpy as np

B, T, DE, H = 32, 2048, 1024, 1024
NCORES = 8
BPC = B // NCORES  # batches per core
P = 128
NK = DE // P  # 8 K-tiles
NT = T // P  # 16 T-tiles per batch
TCHUNK = 512  # T elements per h_enc load chunk
NCHUNK = T // TCHUNK  # 4
MPC = TCHUNK // P  # 4 T-tiles per chunk
DEFER_CHUNKS = 1  # batch-0 chunks whose dec-add runs after the Wd load

_CACHED = {}

LAST_RESULT = None


def _install_ntff_hook():
    try:
        from antenv.axon_hooks import get_axon_ntff_profile_hook  # noqa: F401

        return
    except ImportError:
        pass
    try:
        from trn_agent_boot.trn_boot import _ntff_profile_via_ctypes
    except ImportError:
        return
    so = "/opt/axon/libaxon_pjrt.so"
    if not os.path.exists(so):
        return
    hook = _ntff_profile_via_ctypes(so)
    mod = types.ModuleType("antenv.axon_hooks")
    mod.get_axon_ntff_profile_hook = lambda: hook
    mod.set_axon_ntff_profile_hook = lambda h: None
    sys.modules["antenv.axon_hooks"] = mod


def _build():
    import concourse.bacc as bacc
    import concourse.mybir as mybir
    from concourse.tile import TileContext

    f32 = mybir.dt.float32
    f32r = mybir.dt.float32r
    ALU = mybir.AluOpType
    ACTF = mybir.ActivationFunctionType

    nc = bacc.Bacc()

    hencT = nc.dram_tensor("hencT", (BPC, DE, T), f32r, kind="ExternalInput")
    hdecT = nc.dram_tensor("hdecT", (DE, BPC), f32r, kind="ExternalInput")
    we_d = nc.dram_tensor("we", (DE, H), f32r, kind="ExternalInput")
    wd_d = nc.dram_tensor("wd", (DE, H), f32r, kind="ExternalInput")
    wcb_d = nc.dram_tensor("wcb", (P, H), f32, kind="ExternalInput")
    bdbe_d = nc.dram_tensor("bdbe", (1, H), f32r, kind="ExternalInput")
    be_d = nc.dram_tensor("be_row", (1, H), f32, kind="ExternalInput")
    bcb_d = nc.dram_tensor("bcb", (P, 1), f32, kind="ExternalInput")
    ones_d = nc.dram_tensor("ones_r", (1, P), f32r, kind="ExternalInput")
    ctx_out = nc.dram_tensor("ctx_out", (BPC, H), f32, kind="ExternalOutput")
    attn_out = nc.dram_tensor("attn_out", (BPC, T), f32, kind="ExternalOutput")

    with TileContext(nc) as tc:
        with (
            tc.tile_pool(name="const", bufs=1) as const,
            tc.tile_pool(name="wpool", bufs=1) as wpool,
            tc.tile_pool(name="henc", bufs=3) as henc,
            tc.tile_pool(name="encp", bufs=NT + 3) as encp_pool,
            tc.tile_pool(name="work", bufs=2) as work,
            tc.tile_pool(name="soft", bufs=2) as soft,
            tc.tile_pool(name="psum", bufs=3, space="PSUM") as psum,
            tc.tile_pool(name="psmall", bufs=1, space="PSUM") as psmall,
            tc.tile_pool(name="pctx", bufs=1, space="PSUM") as pctx,
        ):
            # ---- stage the hot-path loads first: h_enc chunk 0 + We ----
            def load_chunk(b, c):
                he = henc.tile([P, NK * TCHUNK], f32r, tag="henc", name=f"he_{b}_{c}")
                hw = NK // 2
                for q in range(2):
                    nc.sync.dma_start(
                        out=he[:, q * hw * TCHUNK : (q + 1) * hw * TCHUNK].rearrange(
                            "p (k t) -> p k t", k=hw
                        ),
                        in_=hencT[
                            b, q * hw * P : (q + 1) * hw * P,
                            c * TCHUNK : (c + 1) * TCHUNK,
                        ].rearrange("(k p) t -> p k t", p=P),
                    )
                return he

            # First chunk + We, interleaved in dependency order: the first
            # matmul group consumes (he half q, we k) in ascending k, so
            # issue those transfers first — concurrent DMA queues share HBM
            # bandwidth fairly, and anything queued early steals bandwidth
            # from the critical path.
            he_next = henc.tile([P, NK * TCHUNK], f32r, tag="henc", name="he_0_0")
            we_sb = wpool.tile([P, NK * H], f32r)
            hw0 = NK // 2
            for q in range(2):
                nc.sync.dma_start(
                    out=he_next[
                        :, q * hw0 * TCHUNK : (q + 1) * hw0 * TCHUNK
                    ].rearrange("p (k t) -> p k t", k=hw0),
                    in_=hencT[0, q * hw0 * P : (q + 1) * hw0 * P, 0:TCHUNK].rearrange(
                        "(k p) t -> p k t", p=P
                    ),
                )
                # issue the We loads from ScalarE's HWDGE so descriptor
                # issue overlaps the h_enc issue on SyncE at startup
                for k in range(q * hw0, (q + 1) * hw0):
                    nc.scalar.dma_start(
                        out=we_sb[:, k * H : (k + 1) * H],
                        in_=we_d[k * P : (k + 1) * P, :],
                    )
            we_t = [we_sb[:, k * H : (k + 1) * H] for k in range(NK)]

            wcb_sb = const.tile([P, H], f32)
            nc.sync.dma_start(out=wcb_sb[:], in_=wcb_d[:, :])
            bdbe_sb = const.tile([1, H], f32r)
            nc.sync.dma_start(out=bdbe_sb[:], in_=bdbe_d[:, :])
            be_sb = const.tile([1, H], f32)
            nc.sync.dma_start(out=be_sb[:], in_=be_d[:, :])
            bcb_sb = const.tile([P, 1], f32)
            nc.sync.dma_start(out=bcb_sb[:], in_=bcb_d[:, :])
            hd_sb = const.tile([P, NK * BPC], f32r)
            nc.sync.dma_start(
                out=hd_sb[:].rearrange("p (k m) -> p k m", k=NK),
                in_=hdecT[:, :].rearrange("(k p) m -> p k m", p=P),
            )
            ones1x128r = const.tile([1, P], f32r)
            nc.sync.dma_start(out=ones1x128r[:], in_=ones_d[:, :])
            ones1x4r = ones1x128r[0:1, 0:BPC]
            ones128x1 = const.tile([P, 1], f32)
            nc.vector.memset(ones128x1[:], 1.0)
            ones1x128f = const.tile([1, P], f32)
            nc.vector.memset(ones1x128f[:], 1.0)

            state = {"he": he_next, "score": None, "encp": [], "held": []}

            def emit_mm_group(b, c, m, he):
                """16 matmuls -> one [128, 1024] psum tile + enc_p copy."""
                t_idx = c * MPC + m
                ps = psum.tile([P, H], f32, tag="ps", name=f"ps_{b}_{t_idx}")
                for k in range(NK):
                    lhsT = he[:, k * TCHUNK + m * P : k * TCHUNK + (m + 1) * P]
                    for h in range(2):
                        nc.tensor.matmul(
                            ps[:, h * 512 : (h + 1) * 512],
                            lhsT,
                            we_t[k][:, h * 512 : (h + 1) * 512],
                            start=(k == 0),
                            stop=(k == NK - 1),
                        )
                ept = encp_pool.tile([P, H], f32r, tag="encp", name=f"ep_{b}_{t_idx}")
                nc.scalar.copy(ept[:], ps[:])
                state["encp"].append(ept)
                return ps

            def emit_score_tail(b, t_idx, buf):
                """tanh in place, then fused mul+reduce against Wc (in place)."""
                nc.scalar.activation(buf[:], buf[:], ACTF.Tanh)
                nc.vector.scalar_tensor_tensor(
                    out=buf[:],
                    in0=buf[:],
                    scalar=1.0,
                    in1=wcb_sb[:],
                    op0=ALU.bypass,
                    op1=ALU.mult,
                    accum_out=state["score"][:, t_idx : t_idx + 1],
                )

            def emit_exp_cols(b, c):
                """exp(score+bc) for this chunk's 4 columns, + the f32r copy
                the ctx matmuls consume -- spread over the batch so the final
                softmax chain is short."""
                cols = slice(c * MPC, (c + 1) * MPC)
                nc.scalar.activation(
                    state["exp"][:, cols],
                    state["score"][:, cols],
                    ACTF.Exp,
                    bias=bcb_sb[:, 0:1],
                )
                nc.scalar.copy(state["expr"][:, cols], state["exp"][:, cols])

            def emit_chunk(b, c, decb, defer=False):
                he = state["he"]
                nxt = (b, c + 1) if c + 1 < NCHUNK else (b + 1, 0)
                if nxt[0] < BPC:
                    state["he"] = load_chunk(*nxt)
                for m in range(MPC):
                    t_idx = c * MPC + m
                    ps = emit_mm_group(b, c, m, he)
                    if defer:
                        tih = work.tile(
                            [P, H], f32, tag="ti", bufs=DEFER_CHUNKS * MPC,
                            name=f"tih_{b}_{t_idx}",
                        )
                        nc.vector.tensor_copy(tih[:], ps[:])
                        state["held"].append((t_idx, tih))
                    else:
                        ti = work.tile(
                            [P, H], f32, tag="ti", bufs=DEFER_CHUNKS * MPC,
                            name=f"ti_{b}_{t_idx}",
                        )
                        nc.vector.tensor_add(ti[:], ps[:], decb[:])
                        emit_score_tail(b, t_idx, ti)
                if not defer:
                    emit_exp_cols(b, c)

            def emit_decb(b, dec_sb):
                dec_row = work.tile([1, H], f32r, tag="dec_row", bufs=1, name=f"dr_{b}")
                nc.sync.dma_start(out=dec_row[:], in_=dec_sb[b : b + 1, :])
                decb = work.tile([P, H], f32, tag="decb", bufs=1, name=f"db_{b}")
                for h in range(2):
                    ps_bc = psmall.tile(
                        [P, 512], f32, tag="psmall", name=f"pbc_{b}_{h}"
                    )
                    nc.tensor.matmul(
                        ps_bc[:],
                        ones1x128r[:],
                        dec_row[0:1, h * 512 : (h + 1) * 512],
                        start=True,
                        stop=True,
                    )
                    nc.vector.tensor_copy(decb[:, h * 512 : (h + 1) * 512], ps_bc[:])
                return decb

            def emit_softmax_ctx(b, exp_mat, exp_r):
                rowsum = soft.tile([P, 1], f32, tag="rowsum", name=f"rs_{b}")
                nc.vector.tensor_reduce(
                    rowsum[:], exp_mat[:], axis=mybir.AxisListType.X, op=ALU.add
                )
                ps_tot = psmall.tile([1, 1], f32, tag="psmall", name=f"pt_{b}")
                nc.tensor.matmul(
                    ps_tot[:], ones128x1[:], rowsum[:], start=True, stop=True
                )
                inv_sb = soft.tile([1, 1], f32, tag="inv", name=f"inv_{b}")
                nc.vector.reciprocal(inv_sb[:], ps_tot[:])
                ps_inv = psmall.tile([P, 1], f32, tag="psmall", name=f"pi_{b}")
                nc.tensor.matmul(
                    ps_inv[:], ones1x128f[:], inv_sb[:], start=True, stop=True
                )
                invb = soft.tile([P, 1], f32, tag="invb", name=f"ivb_{b}")
                nc.vector.tensor_copy(invb[:], ps_inv[:])
                attn_mat = soft.tile([P, NT], f32, tag="attn", name=f"at_{b}")
                nc.vector.tensor_scalar_mul(attn_mat[:], exp_mat[:], invb[:, 0:1])
                nc.sync.dma_start(
                    out=attn_out[b].rearrange("(n p) -> p n", p=P),
                    in_=attn_mat[:],
                )

                # ctx matmuls consume the UNNORMALIZED exp; the 1/sum scale
                # and the be add fuse into one DVE op on the [1, 512] result.
                encp_tiles = state["encp"][b * NT : (b + 1) * NT]
                ctx_sb = soft.tile([1, H], f32, tag="ctx", bufs=1, name=f"cx_{b}")
                for h in range(2):
                    ps_ctx = pctx.tile([1, 512], f32, tag="pctx", name=f"pcx_{b}_{h}")
                    for i in range(NT):
                        nc.tensor.matmul(
                            ps_ctx[:],
                            exp_r[:, i : i + 1],
                            encp_tiles[i][:, h * 512 : (h + 1) * 512],
                            start=(i == 0),
                            stop=(i == NT - 1),
                        )
                    nc.vector.scalar_tensor_tensor(
                        out=ctx_sb[0:1, h * 512 : (h + 1) * 512],
                        in0=ps_ctx[:],
                        scalar=inv_sb[0:1, 0:1],
                        in1=be_sb[0:1, h * 512 : (h + 1) * 512],
                        op0=ALU.mult,
                        op1=ALU.add,
                    )
                nc.sync.dma_start(out=ctx_out[b : b + 1, :], in_=ctx_sb[:])

            # ================= emission schedule =================
            # dec path first: Wd load + dec_full matmul (DMA-ordered right
            # behind We so decb is ready before batch 0 chunk 1 needs it)
            dec_sb = const.tile([BPC, H], f32r)
            for h in range(2):
                wd_half = henc.tile([P, NK * 512], f32r, tag="henc", name=f"wd_{h}")
                hw = NK // 2
                for q in range(2):
                    nc.sync.dma_start(
                        out=wd_half[
                            :, q * hw * 512 : (q + 1) * hw * 512
                        ].rearrange("p (k t) -> p k t", k=hw),
                        in_=wd_d[
                            q * hw * P : (q + 1) * hw * P, h * 512 : (h + 1) * 512
                        ].rearrange("(k p) t -> p k t", p=P),
                    )
                ps_dec = psmall.tile([BPC, 512], f32, tag="psmall", name=f"pd_{h}")
                for k in range(NK):
                    nc.tensor.matmul(
                        ps_dec[:],
                        hd_sb[:, k * BPC : (k + 1) * BPC],
                        wd_half[:, k * 512 : (k + 1) * 512],
                        start=(k == 0),
                        stop=False,
                    )
                nc.tensor.matmul(
                    ps_dec[:],
                    ones1x4r[:],
                    bdbe_sb[0:1, h * 512 : (h + 1) * 512],
                    start=False,
                    stop=True,
                )
                nc.scalar.copy(dec_sb[:, h * 512 : (h + 1) * 512], ps_dec[:])

            score_mats = [
                soft.tile([P, NT], f32, tag="score", name=f"sc_{b}")
                for b in range(BPC)
            ]
            exp_mats = [
                soft.tile([P, NT], f32, tag="exp", name=f"ex_{b}")
                for b in range(BPC)
            ]
            expr_mats = [
                soft.tile([P, NT], f32r, tag="expr", name=f"exr_{b}")
                for b in range(BPC)
            ]
            state["score"] = score_mats[0]
            state["exp"] = exp_mats[0]
            state["expr"] = expr_mats[0]
            emit_chunk(0, 0, None, defer=True)
            decb = emit_decb(0, dec_sb)
            for c in range(DEFER_CHUNKS, NCHUNK):
                emit_chunk(0, c, decb)
                # drain up to 2 deferred chunk-0 tiles per chunk
                for t_idx, tih in state["held"][:2]:
                    nc.vector.tensor_add(tih[:], tih[:], decb[:])
                    emit_score_tail(0, t_idx, tih)
                was_last = state["held"] and len(state["held"]) <= 2
                state["held"] = state["held"][2:]
                if was_last:
                    emit_exp_cols(0, 0)

            for b in range(1, BPC):
                decb = emit_decb(b, dec_sb)
                state["score"] = score_mats[b]
                state["exp"] = exp_mats[b]
                state["expr"] = expr_mats[b]
                emit_chunk(b, 0, decb)
                # softmax+ctx of the previous batch overlap this batch's mms
                emit_softmax_ctx(b - 1, exp_mats[b - 1], expr_mats[b - 1])
                for c in range(1, NCHUNK):
                    emit_chunk(b, c, decb)
            emit_softmax_ctx(BPC - 1, exp_mats[BPC - 1], expr_mats[BPC - 1])

    nc.compile()
    return nc


def kernel(h_enc, h_dec, We, be, Wd, bd, Wc, bc):
    global LAST_RESULT
    _install_ntff_hook()
    from concourse.bass_utils import run_bass_kernel_spmd

    if "nc" not in _CACHED:
        _CACHED["nc"] = _build()
    nc = _CACHED["nc"]

    h_enc = np.ascontiguousarray(h_enc, dtype=np.float32)
    h_dec = np.ascontiguousarray(h_dec, dtype=np.float32)
    We = np.ascontiguousarray(We, dtype=np.float32)
    Wd = np.ascontiguousarray(Wd, dtype=np.float32)
    be = np.asarray(be, dtype=np.float32).reshape(-1)
    bd = np.asarray(bd, dtype=np.float32).reshape(-1)
    Wc = np.asarray(Wc, dtype=np.float32).reshape(-1)
    bc = np.asarray(bc, dtype=np.float32).reshape(-1)

    wcb = np.ascontiguousarray(np.broadcast_to(Wc[None, :], (P, H)))
    bdbe = np.ascontiguousarray((bd + be)[None, :])
    be_row = np.ascontiguousarray(be[None, :])
    bcb = np.full((P, 1), bc[0], dtype=np.float32)
    ones_r = np.ones((1, P), dtype=np.float32)

    in_maps = []
    for core in range(NCORES):
        sl = slice(core * BPC, (core + 1) * BPC)
        in_maps.append(
            {
                "hencT": np.ascontiguousarray(h_enc[sl].transpose(0, 2, 1)),
                "hdecT": np.ascontiguousarray(h_dec[sl].T),
                "we": We,
                "wd": Wd,
                "wcb": wcb,
                "bdbe": bdbe,
                "be_row": be_row,
                "bcb": bcb,
                "ones_r": ones_r,
            }
        )

    try:
        res = run_bass_kernel_spmd(nc, in_maps, core_ids=list(range(NCORES)))
    except Exception:
        # transient axon-worker failures have been observed; retry once
        res = run_bass_kernel_spmd(nc, in_maps, core_ids=list(range(NCORES)))
    LAST_RESULT = res
    if res.exec_time_ns is not None:
        print(f"HW exec time: {res.exec_time_ns} ns")

    ctx = np.concatenate([r["ctx_out"] for r in res.results], axis=0)
    attn = np.concatenate([r["attn_out"] for r in res.results], axis=0)
    return ctx, attn[:, :, None]


# revision 21
# speedup vs baseline: 1.0177x; 1.0177x over previous
"""Bahdanau attention forward on 8 Trainium2 NeuronCores.

Data-parallel over batch B=32: 4 batches per core, dense weights replicated,
no collectives.  Shapes hardcoded: B=32, T=2048, De=Dd=H=1024.

Math (per batch b):
    enc_p  = h_enc[b] @ We + be                  [T, H]
    dec_p  = h_dec[b] @ Wd + bd                  [H]
    score  = tanh(enc_p + dec_p) @ Wc + bc       [T]
    attn   = softmax(score)                      [T]
    ctx    = sum_t attn[t] * enc_p[t]            [H]

Implementation notes:
  - h_enc is pre-transposed on the host to [De, T] so the contraction dim
    (De) lands on SBUF partitions; tiles of it are the stationary matmul
    operand, We (natural [De, H]) is the moving one -> enc_p in natural
    [T, H] layout.  All large matmuls use float32r (full PE rate, ~1e-4
    rel err; fp32 would be 1/4 rate).
  - be is never added on-chip to enc_p: score gets it through the dec row
    (dec_full = dec_p + bd + be) and ctx gets it analytically at the end
    (sum(attn) == 1 so ctx = ctx_raw + be).
  - bc shifts softmax by a constant; folded into the Exp bias (exact).
  - score[t] = sum_h tanh(...)[t,h] * Wc[h] is a free-dim fused
    multiply+reduce on DVE (scalar_tensor_tensor), with Wc pre-broadcast
    to 128 partitions on the host.
  - softmax needs no max-subtraction: |score| <= sum|Wc| ~ 25, exp is
    safe in fp32.
  - ctx = sum over 16 T-tiles of matmul(lhsT=exp_col[128,1],
    rhs=enc_p_tile[128,512]) on the UNNORMALIZED exp; the 1/sum scale and
    the be add fuse into one DVE op on the [1,512] result.  exp runs
    per-chunk so the final softmax->ctx chain is short.
  - Emission is software-pipelined: softmax+ctx of batch b are emitted
    after batch b+1's first chunk so the PE never head-of-line blocks on
    the softmax chain; the first chunk of batch 0 defers its dec_p add
    so the PE does not wait for the Wd load at startup.
"""

import os
import sys
import types

import numpy as np

B, T, DE, H = 32, 2048, 1024, 1024
NCORES = 8
BPC = B // NCORES  # batches per core
P = 128
NK = DE // P  # 8 K-tiles
NT = T // P  # 16 T-tiles per batch
TCHUNK = 512  # T elements per h_enc load chunk
NCHUNK = T // TCHUNK  # 4
MPC = TCHUNK // P  # 4 T-tiles per chunk
DEFER_CHUNKS = 1  # batch-0 chunks whose dec-add runs after the Wd load

_CACHED = {}

LAST_RESULT = None


def _install_ntff_hook():
    try:
        from antenv.axon_hooks import get_axon_ntff_profile_hook  # noqa: F401

        return
    except ImportError:
        pass
    try:
        from trn_agent_boot.trn_boot import _ntff_profile_via_ctypes
    except ImportError:
        return
    so = "/opt/axon/libaxon_pjrt.so"
    if not os.path.exists(so):
        return
    hook = _ntff_profile_via_ctypes(so)
    mod = types.ModuleType("antenv.axon_hooks")
    mod.get_axon_ntff_profile_hook = lambda: hook
    mod.set_axon_ntff_profile_hook = lambda h: None
    sys.modules["antenv.axon_hooks"] = mod


def _build():
    import concourse.bacc as bacc
    import concourse.mybir as mybir
    from concourse.tile import TileContext

    f32 = mybir.dt.float32
    f32r = mybir.dt.float32r
    ALU = mybir.AluOpType
    ACTF = mybir.ActivationFunctionType

    nc = bacc.Bacc()

    hencT = nc.dram_tensor("hencT", (BPC, DE, T), f32r, kind="ExternalInput")
    hdecT = nc.dram_tensor("hdecT", (DE, BPC), f32r, kind="ExternalInput")
    we_d = nc.dram_tensor("we", (DE, H), f32r, kind="ExternalInput")
    wd_d = nc.dram_tensor("wd", (DE, H), f32r, kind="ExternalInput")
    wcb_d = nc.dram_tensor("wcb", (P, H), f32, kind="ExternalInput")
    bdbe_d = nc.dram_tensor("bdbe", (1, H), f32r, kind="ExternalInput")
    be_d = nc.dram_tensor("be_row", (1, H), f32, kind="ExternalInput")
    bcb_d = nc.dram_tensor("bcb", (P, 1), f32, kind="ExternalInput")
    ones_d = nc.dram_tensor("ones_r", (1, P), f32r, kind="ExternalInput")
    ctx_out = nc.dram_tensor("ctx_out", (BPC, H), f32, kind="ExternalOutput")
    attn_out = nc.dram_tensor("attn_out", (BPC, T), f32, kind="ExternalOutput")

    with TileContext(nc) as tc:
        with (
            tc.tile_pool(name="const", bufs=1) as const,
            tc.tile_pool(name="wpool", bufs=1) as wpool,
            tc.tile_pool(name="henc", bufs=3) as henc,
            tc.tile_pool(name="encp", bufs=NT + 2) as encp_pool,
            tc.tile_pool(name="work", bufs=2) as work,
            tc.tile_pool(name="soft", bufs=2) as soft,
            tc.tile_pool(name="psum", bufs=3, space="PSUM") as psum,
            tc.tile_pool(name="psmall", bufs=1, space="PSUM") as psmall,
            tc.tile_pool(name="pctx", bufs=1, space="PSUM") as pctx,
        ):
            # ---- stage the hot-path loads first: h_enc chunk 0 + We ----
            def load_chunk(b, c):
                he = henc.tile([P, NK * TCHUNK], f32r, tag="henc", name=f"he_{b}_{c}")
                hw = NK // 2
                for q in range(2):
                    nc.sync.dma_start(
                        out=he[:, q * hw * TCHUNK : (q + 1) * hw * TCHUNK].rearrange(
                            "p (k t) -> p k t", k=hw
                        ),
                        in_=hencT[
                            b, q * hw * P : (q + 1) * hw * P,
                            c * TCHUNK : (c + 1) * TCHUNK,
                        ].rearrange("(k p) t -> p k t", p=P),
                    )
                return he

            # First chunk + We, interleaved in dependency order: the first
            # matmul group consumes (he half q, we k) in ascending k, so
            # issue those transfers first — concurrent DMA queues share HBM
            # bandwidth fairly, and anything queued early steals bandwidth
            # from the critical path.
            he_next = henc.tile([P, NK * TCHUNK], f32r, tag="henc", name="he_0_0")
            we_sb = wpool.tile([P, NK * H], f32r)
            hw0 = NK // 2
            for q in range(2):
                nc.sync.dma_start(
                    out=he_next[
                        :, q * hw0 * TCHUNK : (q + 1) * hw0 * TCHUNK
                    ].rearrange("p (k t) -> p k t", k=hw0),
                    in_=hencT[0, q * hw0 * P : (q + 1) * hw0 * P, 0:TCHUNK].rearrange(
                        "(k p) t -> p k t", p=P
                    ),
                )
                # issue the We loads from ScalarE's HWDGE so descriptor
                # issue overlaps the h_enc issue on SyncE at startup
                for k in range(q * hw0, (q + 1) * hw0):
                    nc.scalar.dma_start(
                        out=we_sb[:, k * H : (k + 1) * H],
                        in_=we_d[k * P : (k + 1) * P, :],
                    )
            we_t = [we_sb[:, k * H : (k + 1) * H] for k in range(NK)]

            wcb_sb = const.tile([P, H], f32)
            nc.sync.dma_start(out=wcb_sb[:], in_=wcb_d[:, :])
            bdbe_sb = const.tile([1, H], f32r)
            nc.sync.dma_start(out=bdbe_sb[:], in_=bdbe_d[:, :])
            be_sb = const.tile([1, H], f32)
            nc.sync.dma_start(out=be_sb[:], in_=be_d[:, :])
            bcb_sb = const.tile([P, 1], f32)
            nc.sync.dma_start(out=bcb_sb[:], in_=bcb_d[:, :])
            hd_sb = const.tile([P, NK * BPC], f32r)
            nc.sync.dma_start(
                out=hd_sb[:].rearrange("p (k m) -> p k m", k=NK),
                in_=hdecT[:, :].rearrange("(k p) m -> p k m", p=P),
            )
            ones1x128r = const.tile([1, P], f32r)
            nc.sync.dma_start(out=ones1x128r[:], in_=ones_d[:, :])
            ones1x4r = ones1x128r[0:1, 0:BPC]
            ones128x1 = const.tile([P, 1], f32)
            nc.vector.memset(ones128x1[:], 1.0)
            ones1x128f = const.tile([1, P], f32)
            nc.vector.memset(ones1x128f[:], 1.0)

            state = {"he": he_next, "score": None, "encp": [], "held": [], "acc": {}}

            def emit_mm_group(b, c, m, he):
                """16 matmuls -> one [128, 1024] psum tile + enc_p copy."""
                t_idx = c * MPC + m
                ps = psum.tile([P, H], f32, tag="ps", name=f"ps_{b}_{t_idx}")
                for k in range(NK):
                    lhsT = he[:, k * TCHUNK + m * P : k * TCHUNK + (m + 1) * P]
                    for h in range(2):
                        nc.tensor.matmul(
                            ps[:, h * 512 : (h + 1) * 512],
                            lhsT,
                            we_t[k][:, h * 512 : (h + 1) * 512],
                            start=(k == 0),
                            stop=(k == NK - 1),
                        )
                ept = encp_pool.tile([P, H], f32, tag="encp", name=f"ep_{b}_{t_idx}")
                nc.scalar.copy(ept[:], ps[:])
                state["encp"].append(ept)
                return ps

            def emit_score_tail(b, t_idx, buf):
                """tanh in place, then fused mul+reduce against Wc (in place)."""
                nc.scalar.activation(buf[:], buf[:], ACTF.Tanh)
                nc.vector.scalar_tensor_tensor(
                    out=buf[:],
                    in0=buf[:],
                    scalar=1.0,
                    in1=wcb_sb[:],
                    op0=ALU.bypass,
                    op1=ALU.mult,
                    accum_out=state["score"][:, t_idx : t_idx + 1],
                )

            def emit_exp_cols(b, c):
                """exp(score+bc) for this chunk's 4 columns, then fold the
                chunk's enc_p tiles into the ctx accumulator on DVE:
                acc[p, :] += exp[p, tile] * enc_p_tile[p, :].  Per-partition
                scalars work because T is the partition dim in this layout.
                Spreading this per chunk keeps the final softmax chain short
                and takes the ctx contraction off the PE entirely."""
                cols = slice(c * MPC, (c + 1) * MPC)
                nc.scalar.activation(
                    state["exp"][:, cols],
                    state["score"][:, cols],
                    ACTF.Exp,
                    bias=bcb_sb[:, 0:1],
                )
                first = b not in state["acc"]
                if first:
                    acc = work.tile([P, H], f32, tag="acc", bufs=2, name=f"acc_{b}")
                    state["acc"][b] = acc
                acc = state["acc"][b]
                for j in range(MPC):
                    t = c * MPC + j
                    ept = state["encp"][b * NT + t]
                    col = state["exp"][:, t : t + 1]
                    if first and j == 0:
                        nc.vector.tensor_scalar_mul(acc[:], ept[:], col)
                    else:
                        nc.vector.scalar_tensor_tensor(
                            out=acc[:],
                            in0=ept[:],
                            scalar=col,
                            in1=acc[:],
                            op0=ALU.mult,
                            op1=ALU.add,
                        )

            def emit_chunk(b, c, decb, defer=False):
                he = state["he"]
                nxt = (b, c + 1) if c + 1 < NCHUNK else (b + 1, 0)
                if nxt[0] < BPC:
                    state["he"] = load_chunk(*nxt)
                for m in range(MPC):
                    t_idx = c * MPC + m
                    ps = emit_mm_group(b, c, m, he)
                    if defer:
                        tih = work.tile(
                            [P, H], f32, tag="ti", bufs=DEFER_CHUNKS * MPC,
                            name=f"tih_{b}_{t_idx}",
                        )
                        nc.vector.tensor_copy(tih[:], ps[:])
                        state["held"].append((t_idx, tih))
                    else:
                        ti = work.tile(
                            [P, H], f32, tag="ti", bufs=DEFER_CHUNKS * MPC,
                            name=f"ti_{b}_{t_idx}",
                        )
                        nc.vector.tensor_add(ti[:], ps[:], decb[:])
                        emit_score_tail(b, t_idx, ti)
                if not defer:
                    emit_exp_cols(b, c)

            def emit_decb(b, dec_sb):
                dec_row = work.tile([1, H], f32r, tag="dec_row", bufs=1, name=f"dr_{b}")
                nc.sync.dma_start(out=dec_row[:], in_=dec_sb[b : b + 1, :])
                decb = work.tile([P, H], f32, tag="decb", bufs=1, name=f"db_{b}")
                for h in range(2):
                    ps_bc = psmall.tile(
                        [P, 512], f32, tag="psmall", name=f"pbc_{b}_{h}"
                    )
                    nc.tensor.matmul(
                        ps_bc[:],
                        ones1x128r[:],
                        dec_row[0:1, h * 512 : (h + 1) * 512],
                        start=True,
                        stop=True,
                    )
                    nc.vector.tensor_copy(decb[:, h * 512 : (h + 1) * 512], ps_bc[:])
                return decb

            def emit_softmax_ctx(b, exp_mat):
                rowsum = soft.tile([P, 1], f32, tag="rowsum", name=f"rs_{b}")
                nc.vector.tensor_reduce(
                    rowsum[:], exp_mat[:], axis=mybir.AxisListType.X, op=ALU.add
                )
                ps_tot = psmall.tile([1, 1], f32, tag="psmall", name=f"pt_{b}")
                nc.tensor.matmul(
                    ps_tot[:], ones128x1[:], rowsum[:], start=True, stop=True
                )
                inv_sb = soft.tile([1, 1], f32, tag="inv", name=f"inv_{b}")
                nc.vector.reciprocal(inv_sb[:], ps_tot[:])
                ps_inv = psmall.tile([P, 1], f32, tag="psmall", name=f"pi_{b}")
                nc.tensor.matmul(
                    ps_inv[:], ones1x128f[:], inv_sb[:], start=True, stop=True
                )
                invb = soft.tile([P, 1], f32, tag="invb", name=f"ivb_{b}")
                nc.vector.tensor_copy(invb[:], ps_inv[:])
                attn_mat = soft.tile([P, NT], f32, tag="attn", name=f"at_{b}")
                nc.vector.tensor_scalar_mul(attn_mat[:], exp_mat[:], invb[:, 0:1])
                nc.sync.dma_start(
                    out=attn_out[b].rearrange("(n p) -> p n", p=P),
                    in_=attn_mat[:],
                )

                # acc already holds sum_t exp[t]*enc_p[t, :] per partition;
                # finish with a cross-partition ones-matmul (fp32, tiny) and
                # fuse the 1/sum scale + be add into one DVE op.
                acc = state["acc"][b]
                ctx_sb = soft.tile([1, H], f32, tag="ctx", bufs=1, name=f"cx_{b}")
                for h in range(2):
                    ps_ctx = pctx.tile([1, 512], f32, tag="pctx", name=f"pcx_{b}_{h}")
                    nc.tensor.matmul(
                        ps_ctx[:],
                        ones128x1[:],
                        acc[:, h * 512 : (h + 1) * 512],
                        start=True,
                        stop=True,
                    )
                    nc.vector.scalar_tensor_tensor(
                        out=ctx_sb[0:1, h * 512 : (h + 1) * 512],
                        in0=ps_ctx[:],
                        scalar=inv_sb[0:1, 0:1],
                        in1=be_sb[0:1, h * 512 : (h + 1) * 512],
                        op0=ALU.mult,
                        op1=ALU.add,
                    )
                nc.sync.dma_start(out=ctx_out[b : b + 1, :], in_=ctx_sb[:])

            # ================= emission schedule =================
            # dec path first: Wd load + dec_full matmul (DMA-ordered right
            # behind We so decb is ready before batch 0 chunk 1 needs it)
            dec_sb = const.tile([BPC, H], f32r)
            for h in range(2):
                wd_half = henc.tile([P, NK * 512], f32r, tag="henc", name=f"wd_{h}")
                hw = NK // 2
                for q in range(2):
                    nc.sync.dma_start(
                        out=wd_half[
                            :, q * hw * 512 : (q + 1) * hw * 512
                        ].rearrange("p (k t) -> p k t", k=hw),
                        in_=wd_d[
                            q * hw * P : (q + 1) * hw * P, h * 512 : (h + 1) * 512
                        ].rearrange("(k p) t -> p k t", p=P),
                    )
                ps_dec = psmall.tile([BPC, 512], f32, tag="psmall", name=f"pd_{h}")
                for k in range(NK):
                    nc.tensor.matmul(
                        ps_dec[:],
                        hd_sb[:, k * BPC : (k + 1) * BPC],
                        wd_half[:, k * 512 : (k + 1) * 512],
                        start=(k == 0),
                        stop=False,
                    )
                nc.tensor.matmul(
                    ps_dec[:],
                    ones1x4r[:],
                    bdbe_sb[0:1, h * 512 : (h + 1) * 512],
                    start=False,
                    stop=True,
                )
                nc.scalar.copy(dec_sb[:, h * 512 : (h + 1) * 512], ps_dec[:])

            score_mats = [
                soft.tile([P, NT], f32, tag="score", name=f"sc_{b}")
                for b in range(BPC)
            ]
            exp_mats = [
                soft.tile([P, NT], f32, tag="exp", name=f"ex_{b}")
                for b in range(BPC)
            ]
            state["score"] = score_mats[0]
            state["exp"] = exp_mats[0]
            emit_chunk(0, 0, None, defer=True)
            decb = emit_decb(0, dec_sb)
            for c in range(DEFER_CHUNKS, NCHUNK):
                emit_chunk(0, c, decb)
                # drain up to 2 deferred chunk-0 tiles per chunk
                for t_idx, tih in state["held"][:2]:
                    nc.vector.tensor_add(tih[:], tih[:], decb[:])
                    emit_score_tail(0, t_idx, tih)
                was_last = state["held"] and len(state["held"]) <= 2
                state["held"] = state["held"][2:]
                if was_last:
                    emit_exp_cols(0, 0)

            for b in range(1, BPC):
                decb = emit_decb(b, dec_sb)
                state["score"] = score_mats[b]
                state["exp"] = exp_mats[b]
                emit_chunk(b, 0, decb)
                # softmax+ctx of the previous batch overlap this batch's mms
                emit_softmax_ctx(b - 1, exp_mats[b - 1])
                for c in range(1, NCHUNK):
                    emit_chunk(b, c, decb)
            emit_softmax_ctx(BPC - 1, exp_mats[BPC - 1])

    nc.compile()
    return nc


def kernel(h_enc, h_dec, We, be, Wd, bd, Wc, bc):
    global LAST_RESULT
    _install_ntff_hook()
    from concourse.bass_utils import run_bass_kernel_spmd

    if "nc" not in _CACHED:
        _CACHED["nc"] = _build()
    nc = _CACHED["nc"]

    h_enc = np.ascontiguousarray(h_enc, dtype=np.float32)
    h_dec = np.ascontiguousarray(h_dec, dtype=np.float32)
    We = np.ascontiguousarray(We, dtype=np.float32)
    Wd = np.ascontiguousarray(Wd, dtype=np.float32)
    be = np.asarray(be, dtype=np.float32).reshape(-1)
    bd = np.asarray(bd, dtype=np.float32).reshape(-1)
    Wc = np.asarray(Wc, dtype=np.float32).reshape(-1)
    bc = np.asarray(bc, dtype=np.float32).reshape(-1)

    wcb = np.ascontiguousarray(np.broadcast_to(Wc[None, :], (P, H)))
    bdbe = np.ascontiguousarray((bd + be)[None, :])
    be_row = np.ascontiguousarray(be[None, :])
    bcb = np.full((P, 1), bc[0], dtype=np.float32)
    ones_r = np.ones((1, P), dtype=np.float32)

    in_maps = []
    for core in range(NCORES):
        sl = slice(core * BPC, (core + 1) * BPC)
        in_maps.append(
            {
                "hencT": np.ascontiguousarray(h_enc[sl].transpose(0, 2, 1)),
                "hdecT": np.ascontiguousarray(h_dec[sl].T),
                "we": We,
                "wd": Wd,
                "wcb": wcb,
                "bdbe": bdbe,
                "be_row": be_row,
                "bcb": bcb,
                "ones_r": ones_r,
            }
        )

    try:
        res = run_bass_kernel_spmd(nc, in_maps, core_ids=list(range(NCORES)))
    except Exception:
        # transient axon-worker failures have been observed; retry once
        res = run_bass_kernel_spmd(nc, in_maps, core_ids=list(range(NCORES)))
    LAST_RESULT = res
    if res.exec_time_ns is not None:
        print(f"HW exec time: {res.exec_time_ns} ns")

    ctx = np.concatenate([r["ctx_out"] for r in res.results], axis=0)
    attn = np.concatenate([r["attn_out"] for r in res.results], axis=0)
    return ctx, attn[:, :, None]


# revision 22
# speedup vs baseline: 1.0219x; 1.0042x over previous
"""Bahdanau attention forward on 8 Trainium2 NeuronCores.

Data-parallel over batch B=32: 4 batches per core, dense weights replicated,
no collectives.  Shapes hardcoded: B=32, T=2048, De=Dd=H=1024.

Math (per batch b):
    enc_p  = h_enc[b] @ We + be                  [T, H]
    dec_p  = h_dec[b] @ Wd + bd                  [H]
    score  = tanh(enc_p + dec_p) @ Wc + bc       [T]
    attn   = softmax(score)                      [T]
    ctx    = sum_t attn[t] * enc_p[t]            [H]

Implementation notes:
  - h_enc is pre-transposed on the host to [De, T] so the contraction dim
    (De) lands on SBUF partitions; tiles of it are the stationary matmul
    operand, We (natural [De, H]) is the moving one -> enc_p in natural
    [T, H] layout.  All large matmuls use float32r (full PE rate, ~1e-4
    rel err; fp32 would be 1/4 rate).
  - be is never added on-chip to enc_p: score gets it through the dec row
    (dec_full = dec_p + bd + be) and ctx gets it analytically at the end
    (sum(attn) == 1 so ctx = ctx_raw + be).
  - bc shifts softmax by a constant; folded into the Exp bias (exact).
  - score[t] = sum_h tanh(...)[t,h] * Wc[h] is a free-dim fused
    multiply+reduce on DVE (scalar_tensor_tensor), with Wc pre-broadcast
    to 128 partitions on the host.
  - softmax needs no max-subtraction: |score| <= sum|Wc| ~ 25, exp is
    safe in fp32.
  - ctx = sum over 16 T-tiles of matmul(lhsT=exp_col[128,1],
    rhs=enc_p_tile[128,512]) on the UNNORMALIZED exp; the 1/sum scale and
    the be add fuse into one DVE op on the [1,512] result.  exp runs
    per-chunk so the final softmax->ctx chain is short.
  - Emission is software-pipelined: softmax+ctx of batch b are emitted
    after batch b+1's first chunk so the PE never head-of-line blocks on
    the softmax chain; the first chunk of batch 0 defers its dec_p add
    so the PE does not wait for the Wd load at startup.
"""

import os
import sys
import types

import numpy as np

B, T, DE, H = 32, 2048, 1024, 1024
NCORES = 8
BPC = B // NCORES  # batches per core
P = 128
NK = DE // P  # 8 K-tiles
NT = T // P  # 16 T-tiles per batch
TCHUNK = 512  # T elements per h_enc load chunk
NCHUNK = T // TCHUNK  # 4
MPC = TCHUNK // P  # 4 T-tiles per chunk
DEFER_CHUNKS = 1  # batch-0 chunks whose dec-add runs after the Wd load

_CACHED = {}

LAST_RESULT = None


def _install_ntff_hook():
    try:
        from antenv.axon_hooks import get_axon_ntff_profile_hook  # noqa: F401

        return
    except ImportError:
        pass
    try:
        from trn_agent_boot.trn_boot import _ntff_profile_via_ctypes
    except ImportError:
        return
    so = "/opt/axon/libaxon_pjrt.so"
    if not os.path.exists(so):
        return
    hook = _ntff_profile_via_ctypes(so)
    mod = types.ModuleType("antenv.axon_hooks")
    mod.get_axon_ntff_profile_hook = lambda: hook
    mod.set_axon_ntff_profile_hook = lambda h: None
    sys.modules["antenv.axon_hooks"] = mod


def _build():
    import concourse.bacc as bacc
    import concourse.mybir as mybir
    from concourse.tile import TileContext

    f32 = mybir.dt.float32
    f32r = mybir.dt.float32r
    ALU = mybir.AluOpType
    ACTF = mybir.ActivationFunctionType

    nc = bacc.Bacc()

    hencT = nc.dram_tensor("hencT", (BPC, DE, T), f32r, kind="ExternalInput")
    hdecT = nc.dram_tensor("hdecT", (DE, BPC), f32r, kind="ExternalInput")
    we_d = nc.dram_tensor("we", (DE, H), f32r, kind="ExternalInput")
    wd_d = nc.dram_tensor("wd", (DE, H), f32r, kind="ExternalInput")
    wcb_d = nc.dram_tensor("wcb", (P, H), f32, kind="ExternalInput")
    bdbe_d = nc.dram_tensor("bdbe", (1, H), f32r, kind="ExternalInput")
    be_d = nc.dram_tensor("be_row", (1, H), f32, kind="ExternalInput")
    bcb_d = nc.dram_tensor("bcb", (P, 1), f32, kind="ExternalInput")
    ones_d = nc.dram_tensor("ones_r", (1, P), f32r, kind="ExternalInput")
    ctx_out = nc.dram_tensor("ctx_out", (BPC, H), f32, kind="ExternalOutput")
    attn_out = nc.dram_tensor("attn_out", (BPC, T), f32, kind="ExternalOutput")

    with TileContext(nc) as tc:
        with (
            tc.tile_pool(name="const", bufs=1) as const,
            tc.tile_pool(name="wpool", bufs=1) as wpool,
            tc.tile_pool(name="henc", bufs=3) as henc,
            tc.tile_pool(name="encp", bufs=NT + 2) as encp_pool,
            tc.tile_pool(name="work", bufs=2) as work,
            tc.tile_pool(name="soft", bufs=2) as soft,
            tc.tile_pool(name="psum", bufs=3, space="PSUM") as psum,
            tc.tile_pool(name="psmall", bufs=1, space="PSUM") as psmall,
            tc.tile_pool(name="pctx", bufs=1, space="PSUM") as pctx,
        ):
            # ---- stage the hot-path loads first: h_enc chunk 0 + We ----
            def load_chunk(b, c):
                he = henc.tile([P, NK * TCHUNK], f32r, tag="henc", name=f"he_{b}_{c}")
                hw = NK // 2
                for q in range(2):
                    nc.sync.dma_start(
                        out=he[:, q * hw * TCHUNK : (q + 1) * hw * TCHUNK].rearrange(
                            "p (k t) -> p k t", k=hw
                        ),
                        in_=hencT[
                            b, q * hw * P : (q + 1) * hw * P,
                            c * TCHUNK : (c + 1) * TCHUNK,
                        ].rearrange("(k p) t -> p k t", p=P),
                    )
                return he

            # First chunk + We, interleaved in dependency order: the first
            # matmul group consumes (he half q, we k) in ascending k, so
            # issue those transfers first — concurrent DMA queues share HBM
            # bandwidth fairly, and anything queued early steals bandwidth
            # from the critical path.
            he_next = henc.tile([P, NK * TCHUNK], f32r, tag="henc", name="he_0_0")
            we_sb = wpool.tile([P, NK * H], f32r)
            hw0 = NK // 2
            for q in range(2):
                nc.sync.dma_start(
                    out=he_next[
                        :, q * hw0 * TCHUNK : (q + 1) * hw0 * TCHUNK
                    ].rearrange("p (k t) -> p k t", k=hw0),
                    in_=hencT[0, q * hw0 * P : (q + 1) * hw0 * P, 0:TCHUNK].rearrange(
                        "(k p) t -> p k t", p=P
                    ),
                )
                # issue the We loads from ScalarE's HWDGE so descriptor
                # issue overlaps the h_enc issue on SyncE at startup
                for k in range(q * hw0, (q + 1) * hw0):
                    nc.scalar.dma_start(
                        out=we_sb[:, k * H : (k + 1) * H],
                        in_=we_d[k * P : (k + 1) * P, :],
                    )
            we_t = [we_sb[:, k * H : (k + 1) * H] for k in range(NK)]

            wcb_sb = const.tile([P, H], f32)
            nc.sync.dma_start(out=wcb_sb[:], in_=wcb_d[:, :])
            bdbe_sb = const.tile([1, H], f32r)
            nc.sync.dma_start(out=bdbe_sb[:], in_=bdbe_d[:, :])
            be_sb = const.tile([1, H], f32)
            nc.sync.dma_start(out=be_sb[:], in_=be_d[:, :])
            bcb_sb = const.tile([P, 1], f32)
            nc.sync.dma_start(out=bcb_sb[:], in_=bcb_d[:, :])
            hd_sb = const.tile([P, NK * BPC], f32r)
            nc.sync.dma_start(
                out=hd_sb[:].rearrange("p (k m) -> p k m", k=NK),
                in_=hdecT[:, :].rearrange("(k p) m -> p k m", p=P),
            )
            ones1x128r = const.tile([1, P], f32r)
            nc.sync.dma_start(out=ones1x128r[:], in_=ones_d[:, :])
            ones1x4r = ones1x128r[0:1, 0:BPC]
            ones128x1 = const.tile([P, 1], f32)
            nc.vector.memset(ones128x1[:], 1.0)
            ones1x128f = const.tile([1, P], f32)
            nc.vector.memset(ones1x128f[:], 1.0)

            state = {"he": he_next, "score": None, "encp": [], "held": [], "acc": {}}

            def emit_mm_group(b, c, m, he):
                """16 matmuls -> one [128, 1024] psum tile + enc_p copy."""
                t_idx = c * MPC + m
                ps = psum.tile([P, H], f32, tag="ps", name=f"ps_{b}_{t_idx}")
                for k in range(NK):
                    lhsT = he[:, k * TCHUNK + m * P : k * TCHUNK + (m + 1) * P]
                    for h in range(2):
                        nc.tensor.matmul(
                            ps[:, h * 512 : (h + 1) * 512],
                            lhsT,
                            we_t[k][:, h * 512 : (h + 1) * 512],
                            start=(k == 0),
                            stop=(k == NK - 1),
                        )
                ept = encp_pool.tile([P, H], f32, tag="encp", name=f"ep_{b}_{t_idx}")
                nc.scalar.copy(ept[:], ps[:])
                state["encp"].append(ept)
                return ps

            def emit_score_tail(b, t_idx, buf):
                """Per-tile pipeline after the dec-add: tanh in place, fused
                mul+reduce against Wc into the score column, exp of that
                column, then fold this tile's enc_p into the ctx accumulator
                (acc[p,:] += exp[p,t] * enc_p_t[p,:]).  Doing all of it per
                tile keeps DVE work smooth (no chunk-end bursts holding up
                the PSUM rotation) and makes the end-of-kernel chain short."""
                nc.scalar.activation(buf[:], buf[:], ACTF.Tanh)
                sc_col = state["score"][:, t_idx : t_idx + 1]
                nc.vector.scalar_tensor_tensor(
                    out=buf[:],
                    in0=buf[:],
                    scalar=1.0,
                    in1=wcb_sb[:],
                    op0=ALU.bypass,
                    op1=ALU.mult,
                    accum_out=sc_col,
                )
                ex_col = state["exp"][:, t_idx : t_idx + 1]
                nc.scalar.activation(ex_col, sc_col, ACTF.Exp, bias=bcb_sb[:, 0:1])
                ept = state["encp"][b * NT + t_idx]
                first = b not in state["acc"]
                if first:
                    acc = work.tile([P, H], f32, tag="acc", bufs=2, name=f"acc_{b}")
                    state["acc"][b] = acc
                acc = state["acc"][b]
                if first:
                    nc.vector.tensor_scalar_mul(acc[:], ept[:], ex_col)
                else:
                    nc.vector.scalar_tensor_tensor(
                        out=acc[:],
                        in0=ept[:],
                        scalar=ex_col,
                        in1=acc[:],
                        op0=ALU.mult,
                        op1=ALU.add,
                    )

            def emit_chunk(b, c, decb, defer=False):
                he = state["he"]
                nxt = (b, c + 1) if c + 1 < NCHUNK else (b + 1, 0)
                if nxt[0] < BPC:
                    state["he"] = load_chunk(*nxt)
                for m in range(MPC):
                    t_idx = c * MPC + m
                    ps = emit_mm_group(b, c, m, he)
                    if defer:
                        tih = work.tile(
                            [P, H], f32, tag="ti", bufs=DEFER_CHUNKS * MPC,
                            name=f"tih_{b}_{t_idx}",
                        )
                        nc.vector.tensor_copy(tih[:], ps[:])
                        state["held"].append((t_idx, tih))
                    else:
                        ti = work.tile(
                            [P, H], f32, tag="ti", bufs=DEFER_CHUNKS * MPC,
                            name=f"ti_{b}_{t_idx}",
                        )
                        nc.vector.tensor_add(ti[:], ps[:], decb[:])
                        emit_score_tail(b, t_idx, ti)

            def emit_decb(b, dec_sb):
                dec_row = work.tile([1, H], f32r, tag="dec_row", bufs=1, name=f"dr_{b}")
                nc.sync.dma_start(out=dec_row[:], in_=dec_sb[b : b + 1, :])
                decb = work.tile([P, H], f32, tag="decb", bufs=1, name=f"db_{b}")
                for h in range(2):
                    ps_bc = psmall.tile(
                        [P, 512], f32, tag="psmall", name=f"pbc_{b}_{h}"
                    )
                    nc.tensor.matmul(
                        ps_bc[:],
                        ones1x128r[:],
                        dec_row[0:1, h * 512 : (h + 1) * 512],
                        start=True,
                        stop=True,
                    )
                    nc.vector.tensor_copy(decb[:, h * 512 : (h + 1) * 512], ps_bc[:])
                return decb

            def emit_softmax_ctx(b, exp_mat):
                rowsum = soft.tile([P, 1], f32, tag="rowsum", name=f"rs_{b}")
                nc.vector.tensor_reduce(
                    rowsum[:], exp_mat[:], axis=mybir.AxisListType.X, op=ALU.add
                )
                ps_tot = psmall.tile([1, 1], f32, tag="psmall", name=f"pt_{b}")
                nc.tensor.matmul(
                    ps_tot[:], ones128x1[:], rowsum[:], start=True, stop=True
                )
                inv_sb = soft.tile([1, 1], f32, tag="inv", name=f"inv_{b}")
                nc.vector.reciprocal(inv_sb[:], ps_tot[:])
                ps_inv = psmall.tile([P, 1], f32, tag="psmall", name=f"pi_{b}")
                nc.tensor.matmul(
                    ps_inv[:], ones1x128f[:], inv_sb[:], start=True, stop=True
                )
                invb = soft.tile([P, 1], f32, tag="invb", name=f"ivb_{b}")
                nc.vector.tensor_copy(invb[:], ps_inv[:])
                attn_mat = soft.tile([P, NT], f32, tag="attn", name=f"at_{b}")
                nc.vector.tensor_scalar_mul(attn_mat[:], exp_mat[:], invb[:, 0:1])
                nc.sync.dma_start(
                    out=attn_out[b].rearrange("(n p) -> p n", p=P),
                    in_=attn_mat[:],
                )

                # acc already holds sum_t exp[t]*enc_p[t, :] per partition;
                # finish with a cross-partition ones-matmul (fp32, tiny) and
                # fuse the 1/sum scale + be add into one DVE op.
                acc = state["acc"][b]
                ctx_sb = soft.tile([1, H], f32, tag="ctx", bufs=1, name=f"cx_{b}")
                for h in range(2):
                    ps_ctx = pctx.tile([1, 512], f32, tag="pctx", name=f"pcx_{b}_{h}")
                    nc.tensor.matmul(
                        ps_ctx[:],
                        ones128x1[:],
                        acc[:, h * 512 : (h + 1) * 512],
                        start=True,
                        stop=True,
                    )
                    nc.vector.scalar_tensor_tensor(
                        out=ctx_sb[0:1, h * 512 : (h + 1) * 512],
                        in0=ps_ctx[:],
                        scalar=inv_sb[0:1, 0:1],
                        in1=be_sb[0:1, h * 512 : (h + 1) * 512],
                        op0=ALU.mult,
                        op1=ALU.add,
                    )
                nc.sync.dma_start(out=ctx_out[b : b + 1, :], in_=ctx_sb[:])

            # ================= emission schedule =================
            # dec path first: Wd load + dec_full matmul (DMA-ordered right
            # behind We so decb is ready before batch 0 chunk 1 needs it)
            dec_sb = const.tile([BPC, H], f32r)
            for h in range(2):
                wd_half = henc.tile([P, NK * 512], f32r, tag="henc", name=f"wd_{h}")
                hw = NK // 2
                for q in range(2):
                    nc.sync.dma_start(
                        out=wd_half[
                            :, q * hw * 512 : (q + 1) * hw * 512
                        ].rearrange("p (k t) -> p k t", k=hw),
                        in_=wd_d[
                            q * hw * P : (q + 1) * hw * P, h * 512 : (h + 1) * 512
                        ].rearrange("(k p) t -> p k t", p=P),
                    )
                ps_dec = psmall.tile([BPC, 512], f32, tag="psmall", name=f"pd_{h}")
                for k in range(NK):
                    nc.tensor.matmul(
                        ps_dec[:],
                        hd_sb[:, k * BPC : (k + 1) * BPC],
                        wd_half[:, k * 512 : (k + 1) * 512],
                        start=(k == 0),
                        stop=False,
                    )
                nc.tensor.matmul(
                    ps_dec[:],
                    ones1x4r[:],
                    bdbe_sb[0:1, h * 512 : (h + 1) * 512],
                    start=False,
                    stop=True,
                )
                nc.scalar.copy(dec_sb[:, h * 512 : (h + 1) * 512], ps_dec[:])

            score_mats = [
                soft.tile([P, NT], f32, tag="score", name=f"sc_{b}")
                for b in range(BPC)
            ]
            exp_mats = [
                soft.tile([P, NT], f32, tag="exp", name=f"ex_{b}")
                for b in range(BPC)
            ]
            state["score"] = score_mats[0]
            state["exp"] = exp_mats[0]
            emit_chunk(0, 0, None, defer=True)
            decb = emit_decb(0, dec_sb)
            for c in range(DEFER_CHUNKS, NCHUNK):
                emit_chunk(0, c, decb)
                # drain up to 2 deferred chunk-0 tiles per chunk
                for t_idx, tih in state["held"][:2]:
                    nc.vector.tensor_add(tih[:], tih[:], decb[:])
                    emit_score_tail(0, t_idx, tih)
                state["held"] = state["held"][2:]

            for b in range(1, BPC):
                decb = emit_decb(b, dec_sb)
                state["score"] = score_mats[b]
                state["exp"] = exp_mats[b]
                emit_chunk(b, 0, decb)
                # softmax+ctx of the previous batch overlap this batch's mms
                emit_softmax_ctx(b - 1, exp_mats[b - 1])
                for c in range(1, NCHUNK):
                    emit_chunk(b, c, decb)
            emit_softmax_ctx(BPC - 1, exp_mats[BPC - 1])

    nc.compile()
    return nc


def kernel(h_enc, h_dec, We, be, Wd, bd, Wc, bc):
    global LAST_RESULT
    _install_ntff_hook()
    from concourse.bass_utils import run_bass_kernel_spmd

    if "nc" not in _CACHED:
        _CACHED["nc"] = _build()
    nc = _CACHED["nc"]

    h_enc = np.ascontiguousarray(h_enc, dtype=np.float32)
    h_dec = np.ascontiguousarray(h_dec, dtype=np.float32)
    We = np.ascontiguousarray(We, dtype=np.float32)
    Wd = np.ascontiguousarray(Wd, dtype=np.float32)
    be = np.asarray(be, dtype=np.float32).reshape(-1)
    bd = np.asarray(bd, dtype=np.float32).reshape(-1)
    Wc = np.asarray(Wc, dtype=np.float32).reshape(-1)
    bc = np.asarray(bc, dtype=np.float32).reshape(-1)

    wcb = np.ascontiguousarray(np.broadcast_to(Wc[None, :], (P, H)))
    bdbe = np.ascontiguousarray((bd + be)[None, :])
    be_row = np.ascontiguousarray(be[None, :])
    bcb = np.full((P, 1), bc[0], dtype=np.float32)
    ones_r = np.ones((1, P), dtype=np.float32)

    in_maps = []
    for core in range(NCORES):
        sl = slice(core * BPC, (core + 1) * BPC)
        in_maps.append(
            {
                "hencT": np.ascontiguousarray(h_enc[sl].transpose(0, 2, 1)),
                "hdecT": np.ascontiguousarray(h_dec[sl].T),
                "we": We,
                "wd": Wd,
                "wcb": wcb,
                "bdbe": bdbe,
                "be_row": be_row,
                "bcb": bcb,
                "ones_r": ones_r,
            }
        )

    try:
        res = run_bass_kernel_spmd(nc, in_maps, core_ids=list(range(NCORES)))
    except Exception:
        # transient axon-worker failures have been observed; retry once
        res = run_bass_kernel_spmd(nc, in_maps, core_ids=list(range(NCORES)))
    LAST_RESULT = res
    if res.exec_time_ns is not None:
        print(f"HW exec time: {res.exec_time_ns} ns")

    ctx = np.concatenate([r["ctx_out"] for r in res.results], axis=0)
    attn = np.concatenate([r["attn_out"] for r in res.results], axis=0)
    return ctx, attn[:, :, None]


# revision 25
# speedup vs baseline: 1.0299x; 1.0078x over previous
"""Bahdanau attention forward on 8 Trainium2 NeuronCores.

Data-parallel over batch B=32: 4 batches per core, dense weights replicated,
no collectives.  Shapes hardcoded: B=32, T=2048, De=Dd=H=1024.

Math (per batch b):
    enc_p  = h_enc[b] @ We + be                  [T, H]
    dec_p  = h_dec[b] @ Wd + bd                  [H]
    score  = tanh(enc_p + dec_p) @ Wc + bc       [T]
    attn   = softmax(score)                      [T]
    ctx    = sum_t attn[t] * enc_p[t]            [H]

Implementation notes:
  - h_enc is pre-transposed on the host to [De, T] so the contraction dim
    (De) lands on SBUF partitions; tiles of it are the stationary matmul
    operand, We (natural [De, H]) is the moving one -> enc_p in natural
    [T, H] layout.  All large matmuls use float32r (full PE rate, ~1e-4
    rel err; fp32 would be 1/4 rate).
  - be is never added on-chip to enc_p: score gets it through the dec row
    (dec_full = dec_p + bd + be) and ctx gets it analytically at the end
    (sum(attn) == 1 so ctx = ctx_raw + be).
  - bc shifts softmax by a constant; folded into the Exp bias (exact).
  - score[t] = sum_h tanh(...)[t,h] * Wc[h] is a free-dim fused
    multiply+reduce on DVE (scalar_tensor_tensor), with Wc pre-broadcast
    to 128 partitions on the host.
  - softmax needs no max-subtraction: |score| <= sum|Wc| ~ 25, exp is
    safe in fp32.
  - ctx runs on DVE, per tile, fused into the score tail: right after a
    tile's score column is reduced, exp of that column is taken (ACT) and
    acc[p,:] += exp[p,t] * enc_p_t[p,:] accumulates (per-partition scalars
    work because T is the partition dim).  The cross-partition finish is
    one tiny fp32 ones-matmul per half; the 1/sum scale + be add fuse into
    one DVE op.  This takes the whole ctx contraction off the PE (the
    bottleneck engine) and leaves PE and DVE near-balanced (~268us/~250us
    busy).
  - Emission is software-pipelined: softmax+ctx of batch b are emitted
    after batch b+1's first chunk so the PE never head-of-line blocks on
    the softmax chain; the first chunk of batch 0 defers its dec_p add
    so the PE does not wait for the Wd load at startup.
"""

import os
import sys
import types

import numpy as np

B, T, DE, H = 32, 2048, 1024, 1024
NCORES = 8
BPC = B // NCORES  # batches per core
P = 128
NK = DE // P  # 8 K-tiles
NT = T // P  # 16 T-tiles per batch
TCHUNK = 512  # T elements per h_enc load chunk
NCHUNK = T // TCHUNK  # 4
MPC = TCHUNK // P  # 4 T-tiles per chunk
DEFER_CHUNKS = 1  # batch-0 chunks whose dec-add runs after the Wd load

_CACHED = {}

LAST_RESULT = None


def _install_ntff_hook():
    try:
        from antenv.axon_hooks import get_axon_ntff_profile_hook  # noqa: F401

        return
    except ImportError:
        pass
    try:
        from trn_agent_boot.trn_boot import _ntff_profile_via_ctypes
    except ImportError:
        return
    so = "/opt/axon/libaxon_pjrt.so"
    if not os.path.exists(so):
        return
    hook = _ntff_profile_via_ctypes(so)
    mod = types.ModuleType("antenv.axon_hooks")
    mod.get_axon_ntff_profile_hook = lambda: hook
    mod.set_axon_ntff_profile_hook = lambda h: None
    sys.modules["antenv.axon_hooks"] = mod


def _build():
    import concourse.bacc as bacc
    import concourse.mybir as mybir
    from concourse.tile import TileContext

    f32 = mybir.dt.float32
    f32r = mybir.dt.float32r
    ALU = mybir.AluOpType
    ACTF = mybir.ActivationFunctionType

    nc = bacc.Bacc()

    hencT = nc.dram_tensor("hencT", (BPC, DE, T), f32r, kind="ExternalInput")
    hdecT = nc.dram_tensor("hdecT", (DE, BPC), f32r, kind="ExternalInput")
    we_d = nc.dram_tensor("we", (DE, H), f32r, kind="ExternalInput")
    wd_d = nc.dram_tensor("wd", (DE, H), f32r, kind="ExternalInput")
    wcb_d = nc.dram_tensor("wcb", (P, H), f32, kind="ExternalInput")
    bdbe_d = nc.dram_tensor("bdbe", (1, H), f32r, kind="ExternalInput")
    be_d = nc.dram_tensor("be_row", (1, H), f32, kind="ExternalInput")
    bcb_d = nc.dram_tensor("bcb", (P, 1), f32, kind="ExternalInput")
    ones_d = nc.dram_tensor("ones_r", (1, P), f32r, kind="ExternalInput")
    ctx_out = nc.dram_tensor("ctx_out", (BPC, H), f32, kind="ExternalOutput")
    attn_out = nc.dram_tensor("attn_out", (BPC, T), f32, kind="ExternalOutput")

    with TileContext(nc) as tc:
        with (
            tc.tile_pool(name="const", bufs=1) as const,
            tc.tile_pool(name="wpool", bufs=1) as wpool,
            tc.tile_pool(name="henc", bufs=3) as henc,
            tc.tile_pool(name="encp", bufs=NT + 2) as encp_pool,
            tc.tile_pool(name="work", bufs=2) as work,
            tc.tile_pool(name="soft", bufs=2) as soft,
            tc.tile_pool(name="psum", bufs=3, space="PSUM") as psum,
            tc.tile_pool(name="psmall", bufs=1, space="PSUM") as psmall,
            tc.tile_pool(name="pctx", bufs=1, space="PSUM") as pctx,
        ):
            # ---- stage the hot-path loads first: h_enc chunk 0 + We ----
            def load_chunk(b, c):
                he = henc.tile([P, NK * TCHUNK], f32r, tag="henc", name=f"he_{b}_{c}")
                hw = NK // 2
                for q in range(2):
                    nc.sync.dma_start(
                        out=he[:, q * hw * TCHUNK : (q + 1) * hw * TCHUNK].rearrange(
                            "p (k t) -> p k t", k=hw
                        ),
                        in_=hencT[
                            b, q * hw * P : (q + 1) * hw * P,
                            c * TCHUNK : (c + 1) * TCHUNK,
                        ].rearrange("(k p) t -> p k t", p=P),
                    )
                return he

            # First chunk + We, interleaved in dependency order: the first
            # matmul group consumes (he half q, we k) in ascending k, so
            # issue those transfers first — concurrent DMA queues share HBM
            # bandwidth fairly, and anything queued early steals bandwidth
            # from the critical path.
            he_next = henc.tile([P, NK * TCHUNK], f32r, tag="henc", name="he_0_0")
            we_sb = wpool.tile([P, NK * H], f32r)
            for q in range(4):
                qk = NK // 4
                nc.sync.dma_start(
                    out=he_next[
                        :, q * qk * TCHUNK : (q + 1) * qk * TCHUNK
                    ].rearrange("p (k t) -> p k t", k=qk),
                    in_=hencT[0, q * qk * P : (q + 1) * qk * P, 0:TCHUNK].rearrange(
                        "(k p) t -> p k t", p=P
                    ),
                )
                # issue the We loads from ScalarE's HWDGE so descriptor
                # issue overlaps the h_enc issue on SyncE at startup
                for k in range(q * qk, (q + 1) * qk):
                    nc.scalar.dma_start(
                        out=we_sb[:, k * H : (k + 1) * H],
                        in_=we_d[k * P : (k + 1) * P, :],
                    )
            we_t = [we_sb[:, k * H : (k + 1) * H] for k in range(NK)]

            wcb_sb = const.tile([P, H], f32)
            nc.sync.dma_start(out=wcb_sb[:], in_=wcb_d[:, :])
            bdbe_sb = const.tile([1, H], f32r)
            nc.sync.dma_start(out=bdbe_sb[:], in_=bdbe_d[:, :])
            be_sb = const.tile([1, H], f32)
            nc.sync.dma_start(out=be_sb[:], in_=be_d[:, :])
            bcb_sb = const.tile([P, 1], f32)
            nc.sync.dma_start(out=bcb_sb[:], in_=bcb_d[:, :])
            hd_sb = const.tile([P, NK * BPC], f32r)
            nc.sync.dma_start(
                out=hd_sb[:].rearrange("p (k m) -> p k m", k=NK),
                in_=hdecT[:, :].rearrange("(k p) m -> p k m", p=P),
            )
            ones1x128r = const.tile([1, P], f32r)
            nc.sync.dma_start(out=ones1x128r[:], in_=ones_d[:, :])
            ones128x1r = const.tile([P, 1], f32r)
            nc.sync.dma_start(out=ones128x1r[:], in_=ones_d[:, :].rearrange("o p -> p o"))
            ones1x4r = ones1x128r[0:1, 0:BPC]
            ones128x1 = const.tile([P, 1], f32)
            nc.vector.memset(ones128x1[:], 1.0)
            ones1x128f = const.tile([1, P], f32)
            nc.vector.memset(ones1x128f[:], 1.0)

            state = {"he": he_next, "score": None, "encp": [], "held": [], "acc": {}}

            def emit_mm_group(b, c, m, he):
                """16 matmuls -> one [128, 1024] psum tile + enc_p copy."""
                t_idx = c * MPC + m
                ps = psum.tile([P, H], f32, tag="ps", name=f"ps_{b}_{t_idx}")
                for k in range(NK):
                    lhsT = he[:, k * TCHUNK + m * P : k * TCHUNK + (m + 1) * P]
                    for h in range(2):
                        nc.tensor.matmul(
                            ps[:, h * 512 : (h + 1) * 512],
                            lhsT,
                            we_t[k][:, h * 512 : (h + 1) * 512],
                            start=(k == 0),
                            stop=(k == NK - 1),
                        )
                ept = encp_pool.tile([P, H], f32, tag="encp", name=f"ep_{b}_{t_idx}")
                nc.scalar.copy(ept[:], ps[:])
                state["encp"].append(ept)
                return ps

            def emit_score_tail(b, t_idx, buf):
                """Per-tile pipeline after the dec-add: tanh in place, fused
                mul+reduce against Wc into the score column, exp of that
                column, then fold this tile's enc_p into the ctx accumulator
                (acc[p,:] += exp[p,t] * enc_p_t[p,:]).  Doing all of it per
                tile keeps DVE work smooth (no chunk-end bursts holding up
                the PSUM rotation) and makes the end-of-kernel chain short."""
                nc.scalar.activation(buf[:], buf[:], ACTF.Tanh)
                sc_col = state["score"][:, t_idx : t_idx + 1]
                nc.vector.scalar_tensor_tensor(
                    out=buf[:],
                    in0=buf[:],
                    scalar=1.0,
                    in1=wcb_sb[:],
                    op0=ALU.bypass,
                    op1=ALU.mult,
                    accum_out=sc_col,
                )
                ex_col = state["exp"][:, t_idx : t_idx + 1]
                nc.scalar.activation(ex_col, sc_col, ACTF.Exp, bias=bcb_sb[:, 0:1])
                ept = state["encp"][b * NT + t_idx]
                first = b not in state["acc"]
                if first:
                    acc = work.tile([P, H], f32, tag="acc", bufs=2, name=f"acc_{b}")
                    state["acc"][b] = acc
                acc = state["acc"][b]
                if first:
                    nc.vector.tensor_scalar_mul(acc[:], ept[:], ex_col)
                else:
                    nc.vector.scalar_tensor_tensor(
                        out=acc[:],
                        in0=ept[:],
                        scalar=ex_col,
                        in1=acc[:],
                        op0=ALU.mult,
                        op1=ALU.add,
                    )

            def emit_chunk(b, c, decb, defer=False):
                he = state["he"]
                nxt = (b, c + 1) if c + 1 < NCHUNK else (b + 1, 0)
                if nxt[0] < BPC:
                    state["he"] = load_chunk(*nxt)
                for m in range(MPC):
                    t_idx = c * MPC + m
                    ps = emit_mm_group(b, c, m, he)
                    if defer:
                        tih = work.tile(
                            [P, H], f32, tag="ti", bufs=DEFER_CHUNKS * MPC,
                            name=f"tih_{b}_{t_idx}",
                        )
                        nc.vector.tensor_copy(tih[:], ps[:])
                        state["held"].append((t_idx, tih))
                    else:
                        ti = work.tile(
                            [P, H], f32, tag="ti", bufs=DEFER_CHUNKS * MPC,
                            name=f"ti_{b}_{t_idx}",
                        )
                        nc.vector.tensor_add(ti[:], ps[:], decb[:])
                        emit_score_tail(b, t_idx, ti)

            def emit_decb(b, dec_sb):
                dec_row = work.tile([1, H], f32r, tag="dec_row", bufs=1, name=f"dr_{b}")
                nc.sync.dma_start(out=dec_row[:], in_=dec_sb[b : b + 1, :])
                decb = work.tile([P, H], f32, tag="decb", bufs=1, name=f"db_{b}")
                for h in range(2):
                    ps_bc = psmall.tile(
                        [P, 512], f32, tag="psmall", name=f"pbc_{b}_{h}"
                    )
                    nc.tensor.matmul(
                        ps_bc[:],
                        ones1x128r[:],
                        dec_row[0:1, h * 512 : (h + 1) * 512],
                        start=True,
                        stop=True,
                    )
                    nc.vector.tensor_copy(decb[:, h * 512 : (h + 1) * 512], ps_bc[:])
                return decb

            def emit_softmax_ctx(b, exp_mat):
                rowsum = soft.tile([P, 1], f32, tag="rowsum", name=f"rs_{b}")
                nc.vector.tensor_reduce(
                    rowsum[:], exp_mat[:], axis=mybir.AxisListType.X, op=ALU.add
                )
                ps_tot = psmall.tile([1, 1], f32, tag="psmall", name=f"pt_{b}")
                nc.tensor.matmul(
                    ps_tot[:], ones128x1[:], rowsum[:], start=True, stop=True
                )
                inv_sb = soft.tile([1, 1], f32, tag="inv", name=f"inv_{b}")
                nc.vector.reciprocal(inv_sb[:], ps_tot[:])
                ps_inv = psmall.tile([P, 1], f32, tag="psmall", name=f"pi_{b}")
                nc.tensor.matmul(
                    ps_inv[:], ones1x128f[:], inv_sb[:], start=True, stop=True
                )
                invb = soft.tile([P, 1], f32, tag="invb", name=f"ivb_{b}")
                nc.vector.tensor_copy(invb[:], ps_inv[:])
                attn_mat = soft.tile([P, NT], f32, tag="attn", name=f"at_{b}")
                nc.vector.tensor_scalar_mul(attn_mat[:], exp_mat[:], invb[:, 0:1])
                nc.sync.dma_start(
                    out=attn_out[b].rearrange("(n p) -> p n", p=P),
                    in_=attn_mat[:],
                )

                # acc already holds sum_t exp[t]*enc_p[t, :] per partition;
                # finish with a cross-partition ones-matmul (fp32, tiny) and
                # fuse the 1/sum scale + be add into one DVE op.
                acc = state["acc"][b]
                acc_r = work.tile(
                    [P, H], f32r, tag="ti", bufs=DEFER_CHUNKS * MPC, name=f"accr_{b}"
                )
                nc.scalar.copy(acc_r[:], acc[:])
                ctx_sb = soft.tile([1, H], f32, tag="ctx", bufs=1, name=f"cx_{b}")
                for h in range(2):
                    ps_ctx = pctx.tile([1, 512], f32, tag="pctx", name=f"pcx_{b}_{h}")
                    nc.tensor.matmul(
                        ps_ctx[:],
                        ones128x1r[:],
                        acc_r[:, h * 512 : (h + 1) * 512],
                        start=True,
                        stop=True,
                    )
                    nc.vector.scalar_tensor_tensor(
                        out=ctx_sb[0:1, h * 512 : (h + 1) * 512],
                        in0=ps_ctx[:],
                        scalar=inv_sb[0:1, 0:1],
                        in1=be_sb[0:1, h * 512 : (h + 1) * 512],
                        op0=ALU.mult,
                        op1=ALU.add,
                    )
                nc.sync.dma_start(out=ctx_out[b : b + 1, :], in_=ctx_sb[:])

            # ================= emission schedule =================
            # dec path first: Wd load + dec_full matmul (DMA-ordered right
            # behind We so decb is ready before batch 0 chunk 1 needs it)
            dec_sb = const.tile([BPC, H], f32r)
            for h in range(2):
                wd_half = henc.tile([P, NK * 512], f32r, tag="henc", name=f"wd_{h}")
                hw = NK // 2
                for q in range(2):
                    nc.sync.dma_start(
                        out=wd_half[
                            :, q * hw * 512 : (q + 1) * hw * 512
                        ].rearrange("p (k t) -> p k t", k=hw),
                        in_=wd_d[
                            q * hw * P : (q + 1) * hw * P, h * 512 : (h + 1) * 512
                        ].rearrange("(k p) t -> p k t", p=P),
                    )
                ps_dec = psmall.tile([BPC, 512], f32, tag="psmall", name=f"pd_{h}")
                for k in range(NK):
                    nc.tensor.matmul(
                        ps_dec[:],
                        hd_sb[:, k * BPC : (k + 1) * BPC],
                        wd_half[:, k * 512 : (k + 1) * 512],
                        start=(k == 0),
                        stop=False,
                    )
                nc.tensor.matmul(
                    ps_dec[:],
                    ones1x4r[:],
                    bdbe_sb[0:1, h * 512 : (h + 1) * 512],
                    start=False,
                    stop=True,
                )
                nc.scalar.copy(dec_sb[:, h * 512 : (h + 1) * 512], ps_dec[:])

            score_mats = [
                soft.tile([P, NT], f32, tag="score", name=f"sc_{b}")
                for b in range(BPC)
            ]
            exp_mats = [
                soft.tile([P, NT], f32, tag="exp", name=f"ex_{b}")
                for b in range(BPC)
            ]
            state["score"] = score_mats[0]
            state["exp"] = exp_mats[0]
            emit_chunk(0, 0, None, defer=True)
            decb = emit_decb(0, dec_sb)
            for c in range(DEFER_CHUNKS, NCHUNK):
                emit_chunk(0, c, decb)
                # drain up to 2 deferred chunk-0 tiles per chunk
                for t_idx, tih in state["held"][:2]:
                    nc.vector.tensor_add(tih[:], tih[:], decb[:])
                    emit_score_tail(0, t_idx, tih)
                state["held"] = state["held"][2:]

            for b in range(1, BPC):
                decb = emit_decb(b, dec_sb)
                state["score"] = score_mats[b]
                state["exp"] = exp_mats[b]
                emit_chunk(b, 0, decb)
                # softmax+ctx of the previous batch overlap this batch's mms
                emit_softmax_ctx(b - 1, exp_mats[b - 1])
                for c in range(1, NCHUNK):
                    emit_chunk(b, c, decb)
            emit_softmax_ctx(BPC - 1, exp_mats[BPC - 1])

    nc.compile()
    return nc


def kernel(h_enc, h_dec, We, be, Wd, bd, Wc, bc):
    global LAST_RESULT
    _install_ntff_hook()
    from concourse.bass_utils import run_bass_kernel_spmd

    if "nc" not in _CACHED:
        _CACHED["nc"] = _build()
    nc = _CACHED["nc"]

    h_enc = np.ascontiguousarray(h_enc, dtype=np.float32)
    h_dec = np.ascontiguousarray(h_dec, dtype=np.float32)
    We = np.ascontiguousarray(We, dtype=np.float32)
    Wd = np.ascontiguousarray(Wd, dtype=np.float32)
    be = np.asarray(be, dtype=np.float32).reshape(-1)
    bd = np.asarray(bd, dtype=np.float32).reshape(-1)
    Wc = np.asarray(Wc, dtype=np.float32).reshape(-1)
    bc = np.asarray(bc, dtype=np.float32).reshape(-1)

    wcb = np.ascontiguousarray(np.broadcast_to(Wc[None, :], (P, H)))
    bdbe = np.ascontiguousarray((bd + be)[None, :])
    be_row = np.ascontiguousarray(be[None, :])
    bcb = np.full((P, 1), bc[0], dtype=np.float32)
    ones_r = np.ones((1, P), dtype=np.float32)

    in_maps = []
    for core in range(NCORES):
        sl = slice(core * BPC, (core + 1) * BPC)
        in_maps.append(
            {
                "hencT": np.ascontiguousarray(h_enc[sl].transpose(0, 2, 1)),
                "hdecT": np.ascontiguousarray(h_dec[sl].T),
                "we": We,
                "wd": Wd,
                "wcb": wcb,
                "bdbe": bdbe,
                "be_row": be_row,
                "bcb": bcb,
                "ones_r": ones_r,
            }
        )

    try:
        res = run_bass_kernel_spmd(nc, in_maps, core_ids=list(range(NCORES)))
    except Exception:
        # transient axon-worker failures have been observed; retry once
        res = run_bass_kernel_spmd(nc, in_maps, core_ids=list(range(NCORES)))
    LAST_RESULT = res
    if res.exec_time_ns is not None:
        print(f"HW exec time: {res.exec_time_ns} ns")

    ctx = np.concatenate([r["ctx_out"] for r in res.results], axis=0)
    attn = np.concatenate([r["attn_out"] for r in res.results], axis=0)
    return ctx, attn[:, :, None]


# revision 27
# speedup vs baseline: 1.0453x; 1.0150x over previous
"""Bahdanau attention forward on 8 Trainium2 NeuronCores.

Data-parallel over batch B=32: 4 batches per core, dense weights replicated,
no collectives.  Shapes hardcoded: B=32, T=2048, De=Dd=H=1024.

Math (per batch b):
    enc_p  = h_enc[b] @ We + be                  [T, H]
    dec_p  = h_dec[b] @ Wd + bd                  [H]
    score  = tanh(enc_p + dec_p) @ Wc + bc       [T]
    attn   = softmax(score)                      [T]
    ctx    = sum_t attn[t] * enc_p[t]            [H]

Implementation notes:
  - h_enc is pre-transposed on the host to [De, T] so the contraction dim
    (De) lands on SBUF partitions; tiles of it are the stationary matmul
    operand, We (natural [De, H]) is the moving one -> enc_p in natural
    [T, H] layout.  All large matmuls use float32r (full PE rate, ~1e-4
    rel err; fp32 would be 1/4 rate).
  - be is never added on-chip to enc_p: score gets it through the dec row
    (dec_full = dec_p + bd + be) and ctx gets it analytically at the end
    (sum(attn) == 1 so ctx = ctx_raw + be).
  - bc shifts softmax by a constant; folded into the Exp bias (exact).
  - score[t] = sum_h tanh(...)[t,h] * Wc[h] is a free-dim fused
    multiply+reduce on DVE (scalar_tensor_tensor), with Wc pre-broadcast
    to 128 partitions on the host.
  - softmax needs no max-subtraction: |score| <= sum|Wc| ~ 25, exp is
    safe in fp32.
  - ctx runs on DVE, per tile, fused into the score tail: right after a
    tile's score column is reduced, exp of that column is taken (ACT) and
    acc[p,:] += exp[p,t] * enc_p_t[p,:] accumulates (per-partition scalars
    work because T is the partition dim).  The cross-partition finish is
    one tiny ones-matmul per half (acc ACT-cast to float32r first, since
    plain fp32 matmuls are quarter-rate); the 1/sum scale + be add fuse
    into one DVE op.  This takes the whole ctx contraction off the PE (the
    bottleneck engine) and leaves PE and DVE balanced (~259us/~249us busy).
  - Emission is software-pipelined: softmax+ctx of batch b are emitted
    after batch b+1's first chunk so the PE never head-of-line blocks on
    the softmax chain; the first chunk of batch 0 defers its dec_p add
    so the PE does not wait for the Wd load at startup.
"""

import os
import sys
import types

import numpy as np

B, T, DE, H = 32, 2048, 1024, 1024
NCORES = 8
BPC = B // NCORES  # batches per core
P = 128
NK = DE // P  # 8 K-tiles
NT = T // P  # 16 T-tiles per batch
TCHUNK = 512  # T elements per h_enc load chunk
NCHUNK = T // TCHUNK  # 4
MPC = TCHUNK // P  # 4 T-tiles per chunk
DEFER_CHUNKS = 1  # batch-0 chunks whose dec-add runs after the Wd load

_CACHED = {}

LAST_RESULT = None


def _install_ntff_hook():
    try:
        from antenv.axon_hooks import get_axon_ntff_profile_hook  # noqa: F401

        return
    except ImportError:
        pass
    try:
        from trn_agent_boot.trn_boot import _ntff_profile_via_ctypes
    except ImportError:
        return
    so = "/opt/axon/libaxon_pjrt.so"
    if not os.path.exists(so):
        return
    hook = _ntff_profile_via_ctypes(so)
    mod = types.ModuleType("antenv.axon_hooks")
    mod.get_axon_ntff_profile_hook = lambda: hook
    mod.set_axon_ntff_profile_hook = lambda h: None
    sys.modules["antenv.axon_hooks"] = mod


def _build():
    import concourse.bacc as bacc
    import concourse.mybir as mybir
    from concourse.tile import TileContext

    f32 = mybir.dt.float32
    f32r = mybir.dt.float32r
    ALU = mybir.AluOpType
    ACTF = mybir.ActivationFunctionType

    nc = bacc.Bacc()

    hencT = nc.dram_tensor("hencT", (BPC, DE, T), f32r, kind="ExternalInput")
    hdecT = nc.dram_tensor("hdecT", (DE, BPC), f32r, kind="ExternalInput")
    we_d = nc.dram_tensor("we", (DE, H), f32r, kind="ExternalInput")
    wd_d = nc.dram_tensor("wd", (DE, H), f32r, kind="ExternalInput")
    wcb_d = nc.dram_tensor("wcb", (P, H), f32, kind="ExternalInput")
    bdbe_d = nc.dram_tensor("bdbe", (1, H), f32r, kind="ExternalInput")
    be_d = nc.dram_tensor("be_row", (1, H), f32, kind="ExternalInput")
    bcb_d = nc.dram_tensor("bcb", (P, 1), f32, kind="ExternalInput")
    ones_d = nc.dram_tensor("ones_r", (1, P), f32r, kind="ExternalInput")
    ctx_out = nc.dram_tensor("ctx_out", (BPC, H), f32, kind="ExternalOutput")
    attn_out = nc.dram_tensor("attn_out", (BPC, T), f32, kind="ExternalOutput")

    with TileContext(nc) as tc:
        with (
            tc.tile_pool(name="const", bufs=1) as const,
            tc.tile_pool(name="wpool", bufs=1) as wpool,
            tc.tile_pool(name="henc", bufs=3) as henc,
            tc.tile_pool(name="encp", bufs=NT + 2) as encp_pool,
            tc.tile_pool(name="work", bufs=2) as work,
            tc.tile_pool(name="soft", bufs=2) as soft,
            tc.tile_pool(name="psum", bufs=3, space="PSUM") as psum,
            tc.tile_pool(name="psmall", bufs=1, space="PSUM") as psmall,
            tc.tile_pool(name="pctx", bufs=1, space="PSUM") as pctx,
        ):
            # ---- stage the hot-path loads first: h_enc chunk 0 + We ----
            def load_chunk(b, c):
                he = henc.tile([P, NK * TCHUNK], f32r, tag="henc", name=f"he_{b}_{c}")
                hw = NK // 2
                for q in range(2):
                    nc.sync.dma_start(
                        out=he[:, q * hw * TCHUNK : (q + 1) * hw * TCHUNK].rearrange(
                            "p (k t) -> p k t", k=hw
                        ),
                        in_=hencT[
                            b, q * hw * P : (q + 1) * hw * P,
                            c * TCHUNK : (c + 1) * TCHUNK,
                        ].rearrange("(k p) t -> p k t", p=P),
                    )
                return he

            # First chunk + We, interleaved in dependency order: the first
            # matmul group consumes (he half q, we k) in ascending k, so
            # issue those transfers first — concurrent DMA queues share HBM
            # bandwidth fairly, and anything queued early steals bandwidth
            # from the critical path.
            he_next = henc.tile([P, NK * TCHUNK], f32r, tag="henc", name="he_0_0")
            we_sb = wpool.tile([P, NK * H], f32r)
            for q in range(4):
                qk = NK // 4
                nc.sync.dma_start(
                    out=he_next[
                        :, q * qk * TCHUNK : (q + 1) * qk * TCHUNK
                    ].rearrange("p (k t) -> p k t", k=qk),
                    in_=hencT[0, q * qk * P : (q + 1) * qk * P, 0:TCHUNK].rearrange(
                        "(k p) t -> p k t", p=P
                    ),
                )
                # issue the We loads from ScalarE's HWDGE so descriptor
                # issue overlaps the h_enc issue on SyncE at startup
                for k in range(q * qk, (q + 1) * qk):
                    nc.scalar.dma_start(
                        out=we_sb[:, k * H : (k + 1) * H],
                        in_=we_d[k * P : (k + 1) * P, :],
                    )
            we_t = [we_sb[:, k * H : (k + 1) * H] for k in range(NK)]

            wcb_sb = const.tile([P, H], f32)
            nc.sync.dma_start(out=wcb_sb[:], in_=wcb_d[:, :])
            bdbe_sb = const.tile([1, H], f32r)
            nc.sync.dma_start(out=bdbe_sb[:], in_=bdbe_d[:, :])
            be_sb = const.tile([1, H], f32)
            nc.sync.dma_start(out=be_sb[:], in_=be_d[:, :])
            bcb_sb = const.tile([P, 1], f32)
            nc.sync.dma_start(out=bcb_sb[:], in_=bcb_d[:, :])
            hd_sb = const.tile([P, NK * BPC], f32r)
            nc.sync.dma_start(
                out=hd_sb[:].rearrange("p (k m) -> p k m", k=NK),
                in_=hdecT[:, :].rearrange("(k p) m -> p k m", p=P),
            )
            ones1x128r = const.tile([1, P], f32r)
            nc.sync.dma_start(out=ones1x128r[:], in_=ones_d[:, :])
            ones128x1r = const.tile([P, 1], f32r)
            nc.sync.dma_start(out=ones128x1r[:], in_=ones_d[:, :].rearrange("o p -> p o"))
            ones1x4r = ones1x128r[0:1, 0:BPC]
            ones128x1 = const.tile([P, 1], f32)
            nc.vector.memset(ones128x1[:], 1.0)
            ones1x128f = const.tile([1, P], f32)
            nc.vector.memset(ones1x128f[:], 1.0)

            state = {"he": he_next, "score": None, "encp": [], "held": [], "acc": {}}

            def emit_mm_group(b, c, m, he, close=True):
                """16 matmuls -> one [128, 1024] psum tile + enc_p copy."""
                t_idx = c * MPC + m
                ps = psum.tile([P, H], f32, tag="ps", name=f"ps_{b}_{t_idx}")
                for k in range(NK):
                    lhsT = he[:, k * TCHUNK + m * P : k * TCHUNK + (m + 1) * P]
                    for h in range(2):
                        nc.tensor.matmul(
                            ps[:, h * 512 : (h + 1) * 512],
                            lhsT,
                            we_t[k][:, h * 512 : (h + 1) * 512],
                            start=(k == 0),
                            stop=(k == NK - 1) and close,
                        )
                ept = encp_pool.tile([P, H], f32, tag="encp", name=f"ep_{b}_{t_idx}")
                nc.scalar.copy(ept[:], ps[:])
                state["encp"].append(ept)
                return ps

            def emit_score_tail(b, t_idx, buf):
                """Per-tile pipeline after the dec-add: tanh in place, fused
                mul+reduce against Wc into the score column, exp of that
                column, then fold this tile's enc_p into the ctx accumulator
                (acc[p,:] += exp[p,t] * enc_p_t[p,:]).  Doing all of it per
                tile keeps DVE work smooth (no chunk-end bursts holding up
                the PSUM rotation) and makes the end-of-kernel chain short."""
                nc.scalar.activation(buf[:], buf[:], ACTF.Tanh)
                sc_col = state["score"][:, t_idx : t_idx + 1]
                nc.vector.scalar_tensor_tensor(
                    out=buf[:],
                    in0=buf[:],
                    scalar=1.0,
                    in1=wcb_sb[:],
                    op0=ALU.bypass,
                    op1=ALU.mult,
                    accum_out=sc_col,
                )
                ex_col = state["exp"][:, t_idx : t_idx + 1]
                nc.scalar.activation(ex_col, sc_col, ACTF.Exp, bias=bcb_sb[:, 0:1])
                ept = state["encp"][b * NT + t_idx]
                first = b not in state["acc"]
                if first:
                    acc = work.tile([P, H], f32, tag="acc", bufs=2, name=f"acc_{b}")
                    state["acc"][b] = acc
                acc = state["acc"][b]
                if first:
                    nc.vector.tensor_scalar_mul(acc[:], ept[:], ex_col)
                else:
                    nc.vector.scalar_tensor_tensor(
                        out=acc[:],
                        in0=ept[:],
                        scalar=ex_col,
                        in1=acc[:],
                        op0=ALU.mult,
                        op1=ALU.add,
                    )

            def emit_pe_dec_tail(b, t_idx, ps):
                """Close the group with a K=1 dec-add matmul (deferred one
                group so the WAR against the enc_p ACT copy is hidden), then
                tanh PSUM -> SBUF.  Used only where DVE drain time matters."""
                dec_row = state["dec_row"]
                for h in range(2):
                    nc.tensor.matmul(
                        ps[:, h * 512 : (h + 1) * 512],
                        ones1x128r[:],
                        dec_row[0:1, h * 512 : (h + 1) * 512],
                        start=False,
                        stop=True,
                    )
                ti = work.tile(
                    [P, H], f32, tag="ti", bufs=DEFER_CHUNKS * MPC,
                    name=f"tid_{b}_{t_idx}",
                )
                nc.scalar.activation(ti[:], ps[:], ACTF.Tanh)
                sc_col = state["score"][:, t_idx : t_idx + 1]
                nc.vector.scalar_tensor_tensor(
                    out=ti[:],
                    in0=ti[:],
                    scalar=1.0,
                    in1=wcb_sb[:],
                    op0=ALU.bypass,
                    op1=ALU.mult,
                    accum_out=sc_col,
                )
                ex_col = state["exp"][:, t_idx : t_idx + 1]
                nc.scalar.activation(ex_col, sc_col, ACTF.Exp, bias=bcb_sb[:, 0:1])
                acc = state["acc"][b]
                nc.vector.scalar_tensor_tensor(
                    out=acc[:],
                    in0=state["encp"][b * NT + t_idx][:],
                    scalar=ex_col,
                    in1=acc[:],
                    op0=ALU.mult,
                    op1=ALU.add,
                )

            def emit_chunk(b, c, decb, defer=False):
                he = state["he"]
                nxt = (b, c + 1) if c + 1 < NCHUNK else (b + 1, 0)
                if nxt[0] < BPC:
                    state["he"] = load_chunk(*nxt)
                if b == BPC - 1 and c == NCHUNK - 1:
                    pending = []
                    for m in range(MPC):
                        t_idx = c * MPC + m
                        ps = emit_mm_group(b, c, m, he, close=False)
                        pending.append((t_idx, ps))
                        if len(pending) >= 2:
                            emit_pe_dec_tail(b, *pending.pop(0))
                    for t_idx, ps in pending:
                        emit_pe_dec_tail(b, t_idx, ps)
                    return
                for m in range(MPC):
                    t_idx = c * MPC + m
                    ps = emit_mm_group(b, c, m, he)
                    if defer:
                        tih = work.tile(
                            [P, H], f32, tag="ti", bufs=DEFER_CHUNKS * MPC,
                            name=f"tih_{b}_{t_idx}",
                        )
                        nc.vector.tensor_copy(tih[:], ps[:])
                        state["held"].append((t_idx, tih))
                    else:
                        ti = work.tile(
                            [P, H], f32, tag="ti", bufs=DEFER_CHUNKS * MPC,
                            name=f"ti_{b}_{t_idx}",
                        )
                        nc.vector.tensor_add(ti[:], ps[:], decb[:])
                        emit_score_tail(b, t_idx, ti)

            def emit_decb(b, dec_sb):
                dec_row = work.tile([1, H], f32r, tag="dec_row", bufs=1, name=f"dr_{b}")
                nc.sync.dma_start(out=dec_row[:], in_=dec_sb[b : b + 1, :])
                state["dec_row"] = dec_row
                decb = work.tile([P, H], f32, tag="decb", bufs=1, name=f"db_{b}")
                for h in range(2):
                    ps_bc = psmall.tile(
                        [P, 512], f32, tag="psmall", name=f"pbc_{b}_{h}"
                    )
                    nc.tensor.matmul(
                        ps_bc[:],
                        ones1x128r[:],
                        dec_row[0:1, h * 512 : (h + 1) * 512],
                        start=True,
                        stop=True,
                    )
                    nc.vector.tensor_copy(decb[:, h * 512 : (h + 1) * 512], ps_bc[:])
                return decb

            def emit_softmax_ctx(b, exp_mat):
                rowsum = soft.tile([P, 1], f32, tag="rowsum", name=f"rs_{b}")
                nc.vector.tensor_reduce(
                    rowsum[:], exp_mat[:], axis=mybir.AxisListType.X, op=ALU.add
                )
                ps_tot = psmall.tile([1, 1], f32, tag="psmall", name=f"pt_{b}")
                nc.tensor.matmul(
                    ps_tot[:], ones128x1[:], rowsum[:], start=True, stop=True
                )
                inv_sb = soft.tile([1, 1], f32, tag="inv", name=f"inv_{b}")
                nc.vector.reciprocal(inv_sb[:], ps_tot[:])
                ps_inv = psmall.tile([P, 1], f32, tag="psmall", name=f"pi_{b}")
                nc.tensor.matmul(
                    ps_inv[:], ones1x128f[:], inv_sb[:], start=True, stop=True
                )
                invb = soft.tile([P, 1], f32, tag="invb", name=f"ivb_{b}")
                nc.vector.tensor_copy(invb[:], ps_inv[:])
                attn_mat = soft.tile([P, NT], f32, tag="attn", name=f"at_{b}")
                nc.vector.tensor_scalar_mul(attn_mat[:], exp_mat[:], invb[:, 0:1])
                nc.sync.dma_start(
                    out=attn_out[b].rearrange("(n p) -> p n", p=P),
                    in_=attn_mat[:],
                )

                # acc already holds sum_t exp[t]*enc_p[t, :] per partition;
                # finish with a cross-partition ones-matmul (fp32, tiny) and
                # fuse the 1/sum scale + be add into one DVE op.
                acc = state["acc"][b]
                acc_r = work.tile(
                    [P, H], f32r, tag="ti", bufs=DEFER_CHUNKS * MPC, name=f"accr_{b}"
                )
                nc.scalar.copy(acc_r[:], acc[:])
                ctx_sb = soft.tile([1, H], f32, tag="ctx", bufs=1, name=f"cx_{b}")
                for h in range(2):
                    ps_ctx = pctx.tile([1, 512], f32, tag="pctx", name=f"pcx_{b}_{h}")
                    nc.tensor.matmul(
                        ps_ctx[:],
                        ones128x1r[:],
                        acc_r[:, h * 512 : (h + 1) * 512],
                        start=True,
                        stop=True,
                    )
                    nc.vector.scalar_tensor_tensor(
                        out=ctx_sb[0:1, h * 512 : (h + 1) * 512],
                        in0=ps_ctx[:],
                        scalar=inv_sb[0:1, 0:1],
                        in1=be_sb[0:1, h * 512 : (h + 1) * 512],
                        op0=ALU.mult,
                        op1=ALU.add,
                    )
                nc.sync.dma_start(out=ctx_out[b : b + 1, :], in_=ctx_sb[:])

            # ================= emission schedule =================
            # dec path first: Wd load + dec_full matmul (DMA-ordered right
            # behind We so decb is ready before batch 0 chunk 1 needs it)
            dec_sb = const.tile([BPC, H], f32r)
            for h in range(2):
                wd_half = henc.tile([P, NK * 512], f32r, tag="henc", name=f"wd_{h}")
                hw = NK // 2
                for q in range(2):
                    nc.sync.dma_start(
                        out=wd_half[
                            :, q * hw * 512 : (q + 1) * hw * 512
                        ].rearrange("p (k t) -> p k t", k=hw),
                        in_=wd_d[
                            q * hw * P : (q + 1) * hw * P, h * 512 : (h + 1) * 512
                        ].rearrange("(k p) t -> p k t", p=P),
                    )
                ps_dec = psmall.tile([BPC, 512], f32, tag="psmall", name=f"pd_{h}")
                for k in range(NK):
                    nc.tensor.matmul(
                        ps_dec[:],
                        hd_sb[:, k * BPC : (k + 1) * BPC],
                        wd_half[:, k * 512 : (k + 1) * 512],
                        start=(k == 0),
                        stop=False,
                    )
                nc.tensor.matmul(
                    ps_dec[:],
                    ones1x4r[:],
                    bdbe_sb[0:1, h * 512 : (h + 1) * 512],
                    start=False,
                    stop=True,
                )
                nc.scalar.copy(dec_sb[:, h * 512 : (h + 1) * 512], ps_dec[:])

            score_mats = [
                soft.tile([P, NT], f32, tag="score", name=f"sc_{b}")
                for b in range(BPC)
            ]
            exp_mats = [
                soft.tile([P, NT], f32, tag="exp", name=f"ex_{b}")
                for b in range(BPC)
            ]
            state["score"] = score_mats[0]
            state["exp"] = exp_mats[0]
            emit_chunk(0, 0, None, defer=True)
            decb = emit_decb(0, dec_sb)
            for c in range(DEFER_CHUNKS, NCHUNK):
                emit_chunk(0, c, decb)
                # drain up to 2 deferred chunk-0 tiles per chunk
                for t_idx, tih in state["held"][:2]:
                    nc.vector.tensor_add(tih[:], tih[:], decb[:])
                    emit_score_tail(0, t_idx, tih)
                state["held"] = state["held"][2:]

            for b in range(1, BPC):
                decb = emit_decb(b, dec_sb)
                state["score"] = score_mats[b]
                state["exp"] = exp_mats[b]
                emit_chunk(b, 0, decb)
                # softmax+ctx of the previous batch overlap this batch's mms
                emit_softmax_ctx(b - 1, exp_mats[b - 1])
                for c in range(1, NCHUNK):
                    emit_chunk(b, c, decb)
            emit_softmax_ctx(BPC - 1, exp_mats[BPC - 1])

    nc.compile()
    return nc


def kernel(h_enc, h_dec, We, be, Wd, bd, Wc, bc):
    global LAST_RESULT
    _install_ntff_hook()
    from concourse.bass_utils import run_bass_kernel_spmd

    if "nc" not in _CACHED:
        _CACHED["nc"] = _build()
    nc = _CACHED["nc"]

    h_enc = np.ascontiguousarray(h_enc, dtype=np.float32)
    h_dec = np.ascontiguousarray(h_dec, dtype=np.float32)
    We = np.ascontiguousarray(We, dtype=np.float32)
    Wd = np.ascontiguousarray(Wd, dtype=np.float32)
    be = np.asarray(be, dtype=np.float32).reshape(-1)
    bd = np.asarray(bd, dtype=np.float32).reshape(-1)
    Wc = np.asarray(Wc, dtype=np.float32).reshape(-1)
    bc = np.asarray(bc, dtype=np.float32).reshape(-1)

    wcb = np.ascontiguousarray(np.broadcast_to(Wc[None, :], (P, H)))
    bdbe = np.ascontiguousarray((bd + be)[None, :])
    be_row = np.ascontiguousarray(be[None, :])
    bcb = np.full((P, 1), bc[0], dtype=np.float32)
    ones_r = np.ones((1, P), dtype=np.float32)

    in_maps = []
    for core in range(NCORES):
        sl = slice(core * BPC, (core + 1) * BPC)
        in_maps.append(
            {
                "hencT": np.ascontiguousarray(h_enc[sl].transpose(0, 2, 1)),
                "hdecT": np.ascontiguousarray(h_dec[sl].T),
                "we": We,
                "wd": Wd,
                "wcb": wcb,
                "bdbe": bdbe,
                "be_row": be_row,
                "bcb": bcb,
                "ones_r": ones_r,
            }
        )

    try:
        res = run_bass_kernel_spmd(nc, in_maps, core_ids=list(range(NCORES)))
    except Exception:
        # transient axon-worker failures have been observed; retry once
        res = run_bass_kernel_spmd(nc, in_maps, core_ids=list(range(NCORES)))
    LAST_RESULT = res
    if res.exec_time_ns is not None:
        print(f"HW exec time: {res.exec_time_ns} ns")

    ctx = np.concatenate([r["ctx_out"] for r in res.results], axis=0)
    attn = np.concatenate([r["attn_out"] for r in res.results], axis=0)
    return ctx, attn[:, :, None]
